# revision 1
# baseline (speedup 1.0000x reference)
"""ExpanderSAGE GNN kernel for 8x Trainium2 NeuronCores (Bass/Tile).

Strategy (graph/data parallel, dst-sharded):
  - 50000 nodes sharded 6250/core (8 cores). Each core owns the edges whose
    dst lands in its shard (sorted into 128-dst "windows").
  - Neighbor aggregation: per 128-edge chunk, dma_gather the src rows from the
    node-major feature table in HBM, build a one-hot (edge -> dst slot) matrix
    with iota+is_equal on DVE, and accumulate onehot.T @ gathered on the PE
    into a per-window PSUM tile.  inv_deg scaling fused into PSUM evacuation.
  - dma_gather indices are int16, so the table is split in two halves
    (idx < HALF gathers from base 0, the rest from base HALF).
  - Dense layers run feature-major (z^T = W^T @ h^T) with the weights as the
    stationary operand; PE transposes (via identity matmul) move data between
    node-major and feature-major layouts.
  - BatchNorm: per-core sum / sum-of-squares partials -> AllReduce(8 cores) ->
    affine+ReLU applied via DVE + ACT.  Layer-1 output is rebuilt node-major
    and AllGather'd to form the next gather table.  Layer 3 pre-projects
    p = h2 @ w3l (47->pad 64 cols) so only 256B rows are gathered; log_softmax
    is computed node-major per window.
"""

import os
import sys

import numpy as np

for _p in ("/opt/trn_rl_repo", os.path.expanduser("~/.axon_site/_ro/trn_rl_repo")):
    if os.path.isdir(_p) and _p not in sys.path:
        sys.path.insert(0, _p)

import concourse.bacc as bacc
import concourse.mybir as mybir
import concourse.tile as tile
from concourse.bass_utils import run_bass_kernel_spmd
from concourse.masks import make_identity

F32 = mybir.dt.float32
F16 = mybir.dt.float16
F8 = mybir.dt.float8e4
F32R = mybir.dt.float32r
I16 = mybir.dt.int16
I32 = mybir.dt.int32
AL = mybir.AluOpType
AF = mybir.ActivationFunctionType

EPS = 1e-5
NCORES = 8
P = 128
BLK = 512  # node block for dense matmuls (fp32r moving-dim sweet spot)

# Set to True to run the segment-sum + dense matmuls in fp32r (fast fp32).
USE_F32R = os.environ.get("USE_F32R", "0") == "1"
# fp16 gather tables for layers 1/2 (halves gather bytes; 1cyc/row PE)
GATHER_F16 = os.environ.get("GATHER_F16", "1") == "1"
# fp8 e4m3 for the layer-2 neighbor table (halves the dominant gather term;
# the self term and all dense math stay f32)
GATHER_F8L2 = os.environ.get("GATHER_F8L2", "0") == "1"


# --------------------------------------------------------------------------
# Host-side preprocessing: shard edges, build gather-index / one-hot inputs
# --------------------------------------------------------------------------

def preprocess(edge_index, n_nodes):
    src = np.asarray(edge_index[0], dtype=np.int64)
    dst = np.asarray(edge_index[1], dtype=np.int64)
    E = src.shape[0]
    S = n_nodes // NCORES
    NW = (S + P - 1) // P
    HALF = n_nodes // 2
    assert HALF < 32768 and n_nodes - HALF < 32768

    deg = np.bincount(dst, minlength=n_nodes).astype(np.float32)
    invdeg = (1.0 / np.maximum(deg, 1.0)).astype(np.float32)

    core = dst // S
    local = dst - core * S
    w = local // P
    slot = local % P
    half = (src >= HALF).astype(np.int64)
    idxval = (src - half * HALF).astype(np.int64)

    # group key (core, window, half); stable sort groups the edges
    key = (core * NW + w) * 2 + half
    order = np.argsort(key, kind="stable")
    skey = key[order]
    sidx = idxval[order]
    sslot = slot[order]

    ngroups = NCORES * NW * 2
    counts = np.bincount(skey, minlength=ngroups).reshape(NCORES, NW, 2)
    starts = np.zeros(ngroups + 1, dtype=np.int64)
    np.cumsum(counts.reshape(-1), out=starts[1:])

    # uniform chunk counts across cores (SPMD: one NEFF for all 8 cores)
    C = np.ceil(counts.max(axis=0) / P).astype(np.int64)  # [NW, 2]
    CTOT = int(C.sum())
    LTOT = CTOT * P

    # per-(w,half) column offsets into the concatenated device arrays
    chunk_off = np.zeros((NW, 2), dtype=np.int64)
    acc = 0
    for wi in range(NW):
        for h in range(2):
            chunk_off[wi, h] = acc
            acc += C[wi, h]

    idx_arrs = []
    dslot_arrs = []
    for c in range(NCORES):
        idx_a = np.zeros((16, LTOT // 16), dtype=np.int16)
        ds_a = np.full((P, CTOT), -1.0, dtype=np.float32)
        for wi in range(NW):
            for h in range(2):
                cwh = int(C[wi, h])
                if cwh == 0:
                    continue
                g = (c * NW + wi) * 2 + h
                s0, s1 = starts[g], starts[g + 1]
                k = s1 - s0
                Lw = cwh * P
                buf = np.zeros(Lw, dtype=np.int16)
                buf[:k] = sidx[s0:s1].astype(np.int16)
                col0 = chunk_off[wi, h] * 8  # int16 cols consumed so far (=L/16)
                idx_a[:, col0 : col0 + Lw // 16] = buf.reshape(-1, 16).T
                sl = np.full(Lw, -1.0, dtype=np.float32)
                sl[:k] = sslot[s0:s1].astype(np.float32)
                ds_a[:, chunk_off[wi, h] : chunk_off[wi, h] + cwh] = sl.reshape(
                    cwh, P
                ).T
        idx_arrs.append(np.tile(idx_a, (8, 1)))  # replicate to 128 partitions
        dslot_arrs.append(ds_a)

    # invdeg per core, [128, NW] (partition = slot, col = window), pad 1.0
    invd_arrs = []
    for c in range(NCORES):
        v = np.ones(NW * P, dtype=np.float32)
        v[:S] = invdeg[c * S : (c + 1) * S]
        invd_arrs.append(v.reshape(NW, P).T.copy())

    meta = dict(N=n_nodes, S=S, NW=NW, HALF=HALF, C=C, chunk_off=chunk_off,
                CTOT=CTOT, ITOT=LTOT // 16)
    return meta, idx_arrs, dslot_arrs, invd_arrs


# --------------------------------------------------------------------------
# Device program
# --------------------------------------------------------------------------

def _mm_dt(ap):
    return ap.bitcast(F32R) if USE_F32R else ap


def build_program(meta, INDIM, HID, OUT, reps=1, ncores=NCORES, mock_cc=False,
                  abl=()):
    N, S, NW, HALF = meta["N"], meta["S"], meta["NW"], meta["HALF"]
    C, chunk_off, CTOT, ITOT = meta["C"], meta["chunk_off"], meta["CTOT"], meta["ITOT"]
    LASTW = S - P * (NW - 1)
    NBLK = (S + BLK - 1) // BLK
    OUTP = 64  # padded projection width for layer 3 (47 -> 64)
    RG = [list(range(ncores))]

    nc = bacc.Bacc("TRN2", target_bir_lowering=False, debug=False,
                   num_devices=ncores, num_swdge_queues=4)
    qctr = [0]

    # ---- I/O ----
    TDT = F16 if GATHER_F16 else F32
    L2DT = F8 if (GATHER_F8L2 and GATHER_F16) else TDT
    xfull = nc.dram_tensor("xfull", [N, INDIM], TDT, kind="ExternalInput")
    xT = nc.dram_tensor("xT", [P, S], F32, kind="ExternalInput")
    idx_d = nc.dram_tensor("idx", [P, ITOT], I16, kind="ExternalInput")
    dslot_d = nc.dram_tensor("dslot", [P, CTOT], F32, kind="ExternalInput")
    invd_d = nc.dram_tensor("invd", [P, NW], F32, kind="ExternalInput")
    w1l_d = nc.dram_tensor("w1l", [INDIM, HID], F32, kind="ExternalInput")
    w1r_d = nc.dram_tensor("w1r", [INDIM, HID], F32, kind="ExternalInput")
    w2l_d = nc.dram_tensor("w2l", [HID, HID], F32, kind="ExternalInput")
    w2r_d = nc.dram_tensor("w2r", [HID, HID], F32, kind="ExternalInput")
    w3l_d = nc.dram_tensor("w3l", [HID, OUTP], F32, kind="ExternalInput")
    w3r_d = nc.dram_tensor("w3r", [HID, OUT], F32, kind="ExternalInput")
    g1_d = nc.dram_tensor("g1", [P, 2], F32, kind="ExternalInput")
    be1_d = nc.dram_tensor("be1", [P, 2], F32, kind="ExternalInput")
    g2_d = nc.dram_tensor("g2", [P, 2], F32, kind="ExternalInput")
    be2_d = nc.dram_tensor("be2", [P, 2], F32, kind="ExternalInput")
    b3_d = nc.dram_tensor("b3", [P, 1], F32, kind="ExternalInput")
    out_d = nc.dram_tensor("out", [S, OUT], F32, kind="ExternalOutput")

    from contextlib import ExitStack

    with tile.TileContext(nc) as tc, ExitStack() as es:
        cp = es.enter_context(tc.tile_pool(name="const", bufs=1))
        gp = es.enter_context(tc.tile_pool(name="gath", bufs=2))
        ohp = es.enter_context(tc.tile_pool(name="oh", bufs=2))
        agp = es.enter_context(tc.tile_pool(name="agg", bufs=4))
        atp = es.enter_context(tc.tile_pool(name="aggT", bufs=4))
        bigp = es.enter_context(tc.tile_pool(name="big", bufs=4))
        sqp = es.enter_context(tc.tile_pool(name="sq", bufs=2))
        smp = es.enter_context(tc.tile_pool(name="small", bufs=2))
        pseg = es.enter_context(tc.tile_pool(name="pseg", bufs=3, space="PSUM"))
        ptr = es.enter_context(tc.tile_pool(name="ptr", bufs=2, space="PSUM"))
        pz = es.enter_context(tc.tile_pool(name="pz", bufs=2, space="PSUM"))
        drp = es.enter_context(tc.tile_pool(name="dram", bufs=1, space="DRAM"))

        # ---- constants ----
        idx_sb = cp.tile([P, ITOT], I16, name="idx_sb")
        nc.sync.dma_start(idx_sb[:], idx_d[:, :])
        dslot_sb = cp.tile([P, CTOT], F32, name="dslot_sb")
        nc.sync.dma_start(dslot_sb[:], dslot_d[:, :])
        invd_sb = cp.tile([P, NW], F32, name="invd_sb")
        nc.sync.dma_start(invd_sb[:], invd_d[:, :])

        w1l_sb = cp.tile([P, HID], F32, name="w1l_sb")
        nc.sync.dma_start(w1l_sb[:], w1l_d[:, :])
        w1r_sb = cp.tile([P, HID], F32, name="w1r_sb")
        nc.sync.dma_start(w1r_sb[:], w1r_d[:, :])
        w2l_sb = [cp.tile([P, HID], F32, name=f"w2l_sb{k}") for k in range(2)]
        w2r_sb = [cp.tile([P, HID], F32, name=f"w2r_sb{k}") for k in range(2)]
        w3l_sb = [cp.tile([P, OUTP], F32, name=f"w3l_sb{k}") for k in range(2)]
        w3r_sb = [cp.tile([P, OUT], F32, name=f"w3r_sb{k}") for k in range(2)]
        for k in range(2):
            nc.sync.dma_start(w2l_sb[k][:], w2l_d[k * P : (k + 1) * P, :])
            nc.sync.dma_start(w2r_sb[k][:], w2r_d[k * P : (k + 1) * P, :])
            nc.sync.dma_start(w3l_sb[k][:], w3l_d[k * P : (k + 1) * P, :])
            nc.sync.dma_start(w3r_sb[k][:], w3r_d[k * P : (k + 1) * P, :])
        g1_sb = cp.tile([P, 2], F32, name="g1_sb")
        nc.sync.dma_start(g1_sb[:], g1_d[:, :])
        be1_sb = cp.tile([P, 2], F32, name="be1_sb")
        nc.sync.dma_start(be1_sb[:], be1_d[:, :])
        g2_sb = cp.tile([P, 2], F32, name="g2_sb")
        nc.sync.dma_start(g2_sb[:], g2_d[:, :])
        be2_sb = cp.tile([P, 2], F32, name="be2_sb")
        nc.sync.dma_start(be2_sb[:], be2_d[:, :])
        b3_sb = cp.tile([P, 1], F32, name="b3_sb")
        nc.sync.dma_start(b3_sb[:], b3_d[:, :])

        iota_i = cp.tile([P, P], I32, name="iota_i")
        nc.gpsimd.iota(iota_i[:], pattern=[[1, P]], base=0, channel_multiplier=0)
        iota_f = cp.tile([P, P], F32, name="iota_f")
        nc.vector.tensor_copy(iota_f[:], iota_i[:])
        iota_h = cp.tile([P, P], F16, name="iota_h")
        nc.vector.tensor_copy(iota_h[:], iota_i[:])
        dslot_h = cp.tile([P, CTOT], F16, name="dslot_h")
        nc.vector.tensor_copy(dslot_h[:], dslot_sb[:])
        ident = cp.tile([P, P], F32, name="ident")
        make_identity(nc, ident[:])
        eps_sb = cp.tile([P, 1], F32, name="eps_sb")
        nc.vector.memset(eps_sb[:], EPS)


        def gather_segsum_window(wi, F, lo_ap, hi_ap, lname, tdt=F32):
            """Gather + one-hot matmul accumulate one window; returns scaled
            agg tile [128, F] (node-major: partition=dst slot)."""
            ctot = int(C[wi, 0] + C[wi, 1])
            aggw = agp.tile([P, F], F32, tag="agg", name=f"agg{lname}_{wi}")
            if ctot == 0:
                nc.vector.memset(aggw[:], 0.0)
                return aggw
            ps = pseg.tile([P, F], F32, tag="seg", name=f"pseg{lname}_{wi}")
            done = 0
            for h, base_ap in ((0, lo_ap), (1, hi_ap)):
                cwh = int(C[wi, h])
                if cwh == 0:
                    continue
                co = int(chunk_off[wi, h])
                g_t = gp.tile([P, cwh * F], tdt, tag="g", name=f"g{lname}_{wi}_{h}")
                gv = g_t[:].rearrange("p (c f) -> p c f", f=F)
                # split into <=MAXC-chunk dma_gather calls (ring capacity),
                # balanced so no call exceeds 6*128=768 indices
                MAXC = int(os.environ.get("MAXC", "6"))
                npieces = -(-cwh // MAXC)
                sizes = [cwh // npieces + (1 if i < cwh % npieces else 0)
                         for i in range(npieces)]
                offs = [sum(sizes[:i]) for i in range(npieces)]
                if "nogather" in abl:
                    nc.vector.memset(g_t[:, 0:1], 0.0)
                if "nogather" not in abl:
                    for c0, cn in zip(offs, sizes):
                        nc.gpsimd.dma_gather(
                            out_ap=gv[:, c0 : c0 + cn, :],
                            in_ap=base_ap,
                            idxs_ap=idx_sb[:, (co + c0) * 8 : (co + c0 + cn) * 8],
                            num_idxs=cn * P,
                            num_idxs_reg=cn * P,
                            elem_size=F,
                            queue_num=qctr[0] % 4,
                        )
                        qctr[0] += 1
                oh_t = ohp.tile([P, cwh * P], tdt, tag="oh",
                                name=f"oh{lname}_{wi}_{h}")
                ohv = oh_t[:].rearrange("p (c q) -> p c q", q=P)
                if "noonehot" in abl:
                    nc.vector.memset(oh_t[:, 0:1], 0.0)
                else:
                    iota_t = iota_f if tdt == F32 else iota_h
                    dslot_t = dslot_sb if tdt == F32 else dslot_h
                    nc.vector.tensor_tensor(
                        out=ohv,
                        in0=iota_t[:].unsqueeze(1).to_broadcast([P, cwh, P]),
                        in1=dslot_t[:, co : co + cwh].unsqueeze(2).to_broadcast(
                            [P, cwh, P]
                        ),
                        op=AL.is_equal,
                    )
                for ch in range(cwh):
                    if "nosegmm" in abl:
                        done += 1
                        continue
                    nc.tensor.matmul(
                        out=ps[:],
                        lhsT=_mm_dt(ohv[:, ch, :]),
                        rhs=_mm_dt(gv[:, ch, :]),
                        start=(done == 0),
                        stop=(done == ctot - 1),
                    )
                    done += 1
            nc.vector.tensor_scalar(
                out=aggw[:], in0=ps[:], scalar1=invd_sb[:, wi : wi + 1],
                scalar2=None, op0=AL.mult,
            )
            return aggw

        def dense_layer(lname, F_in, lo_ap, hi_ap, hT, wl_sb, wr_sb, tdt=F32):
            """Full SAGE layer (aggregate + dense), feature-major output.
            Returns (zT halves, sum partials, sumsq partials)."""
            nh_in = F_in // P
            zT = [bigp.tile([P, S], F32, tag="big", name=f"zT{lname}_{m}")
                  for m in range(2)]
            szp = [smp.tile([P, NBLK], F32, tag=f"szp{lname}{m}",
                            name=f"szp{lname}{m}") for m in range(2)]
            ssqp = [smp.tile([P, NBLK], F32, tag=f"ssqp{lname}{m}",
                             name=f"ssqp{lname}{m}") for m in range(2)]
            for b in range(NBLK):
                n0 = b * BLK
                nb = min(S, n0 + BLK) - n0
                wlist = [wi for wi in range(4 * b, min(4 * b + 4, NW))]
                aggT = [atp.tile([P, BLK], F32, tag="aggT",
                                 name=f"aggT{lname}_{b}_{h}")
                        for h in range(nh_in)]
                for wi in wlist:
                    aggw = gather_segsum_window(wi, F_in, lo_ap, hi_ap, lname, tdt)
                    for h in range(nh_in):
                        pst = ptr.tile([P, P], F32, tag="tr",
                                       name=f"ptr{lname}_{wi}_{h}")
                        nc.tensor.transpose(
                            pst[:], aggw[:, h * P : (h + 1) * P], ident[:]
                        )
                        c0 = (wi - 4 * b) * P
                        cw = min(P, nb - c0)
                        nc.vector.tensor_copy(
                            out=aggT[h][:, c0 : c0 + cw], in_=pst[:, :cw]
                        )
                for m in range(2):
                    psz = pz.tile([P, BLK], F32, tag="z",
                                  name=f"pz{lname}_{b}_{m}")
                    mcols = slice(m * P, (m + 1) * P)
                    nmm = 2 * nh_in
                    k = 0
                    for h in range(nh_in):
                        nc.tensor.matmul(
                            out=psz[:, :nb],
                            lhsT=_mm_dt(wl_sb[h][:, mcols]),
                            rhs=_mm_dt(aggT[h][:, :nb]),
                            start=(k == 0), stop=(k == nmm - 1),
                        )
                        k += 1
                    for h in range(nh_in):
                        nc.tensor.matmul(
                            out=psz[:, :nb],
                            lhsT=_mm_dt(wr_sb[h][:, mcols]),
                            rhs=_mm_dt(hT[h][:, n0 : n0 + nb]),
                            start=(k == 0), stop=(k == nmm - 1),
                        )
                        k += 1
                    nc.vector.tensor_reduce(
                        out=szp[m][:, b : b + 1], in_=psz[:, :nb],
                        axis=mybir.AxisListType.X, op=AL.add,
                    )
                    sqsc = sqp.tile([P, BLK], F32, tag="sq",
                                    name=f"sq{lname}_{b}_{m}")
                    nc.scalar.activation(
                        out=sqsc[:, :nb], in_=psz[:, :nb], func=AF.Square,
                        accum_out=ssqp[m][:, b : b + 1],
                    )
                    nc.vector.tensor_copy(out=zT[m][:, n0 : n0 + nb],
                                          in_=psz[:, :nb])
            return zT, szp, ssqp

        def collective(kind, op, ins, outs):
            if mock_cc:
                nc.sync.dma_start(outs[0][0 : ins[0].shape[0]], ins[0])
            else:
                nc.gpsimd.collective_compute(kind, op, replica_groups=RG,
                                             ins=ins, outs=outs)

        def bn_relu(li, lname, zT, szp, ssqp, g_sb, be_sb):
            """AllReduce stats, then hT = relu((z - mean) * a + be)."""
            stat = smp.tile([P, 4], F32, tag=f"stat{lname}", name=f"stat{lname}")
            for m in range(2):
                nc.vector.tensor_reduce(out=stat[:, m : m + 1], in_=szp[m][:],
                                        axis=mybir.AxisListType.X, op=AL.add)
                nc.vector.tensor_reduce(out=stat[:, 2 + m : 3 + m],
                                        in_=ssqp[m][:],
                                        axis=mybir.AxisListType.X, op=AL.add)
            nc.sync.dma_start(st_in[li][:], stat[:])
            collective("AllReduce", AL.add, [st_in[li][:]], [st_out[li][:]])
            statg = smp.tile([P, 4], F32, tag=f"statg{lname}", name=f"statg{lname}")
            nc.sync.dma_start(statg[:], st_out[li][:])
            hT = []
            for m in range(2):
                mean = smp.tile([P, 1], F32, tag=f"mean{lname}{m}",
                                name=f"mean{lname}{m}")
                nc.vector.tensor_scalar(out=mean[:], in0=statg[:, m : m + 1],
                                        scalar1=1.0 / N, scalar2=None,
                                        op0=AL.mult)
                ex2 = smp.tile([P, 1], F32, tag=f"ex2{lname}{m}",
                               name=f"ex2{lname}{m}")
                nc.vector.tensor_scalar(out=ex2[:], in0=statg[:, 2 + m : 3 + m],
                                        scalar1=1.0 / N, scalar2=None,
                                        op0=AL.mult)
                nvar = smp.tile([P, 1], F32, tag=f"nvar{lname}{m}",
                                name=f"nvar{lname}{m}")
                # nvar = mean^2 - E[x^2]  (= -var)
                nc.vector.scalar_tensor_tensor(
                    out=nvar[:], in0=mean[:], scalar=mean[:], in1=ex2[:],
                    op0=AL.mult, op1=AL.subtract,
                )
                std = smp.tile([P, 1], F32, tag=f"std{lname}{m}",
                               name=f"std{lname}{m}")
                nc.scalar.activation(out=std[:], in_=nvar[:], func=AF.Sqrt,
                                     bias=eps_sb[:], scale=-1.0)
                istd = smp.tile([P, 1], F32, tag=f"istd{lname}{m}",
                                name=f"istd{lname}{m}")
                nc.vector.reciprocal(istd[:], std[:])
                a_m = smp.tile([P, 1], F32, tag=f"a{lname}{m}",
                               name=f"a{lname}{m}")
                nc.vector.tensor_tensor(out=a_m[:], in0=g_sb[:, m : m + 1],
                                        in1=istd[:], op=AL.mult)
                h_m = bigp.tile([P, S], F32, tag="big", name=f"hT{lname}_{m}")
                nc.vector.tensor_scalar(out=h_m[:], in0=zT[m][:],
                                        scalar1=mean[:], scalar2=a_m[:],
                                        op0=AL.subtract, op1=AL.mult)
                nc.scalar.activation(out=h_m[:], in_=h_m[:], func=AF.Relu,
                                     bias=be_sb[:, m : m + 1], scale=1.0)
                hT.append(h_m)
            return hT

        for rep in range(reps):
            # ---- DRAM intermediates (fresh per rep: Shared tiles are
            # single-writer) ----
            h1_shard = drp.tile([S, HID], L2DT, name=f"h1_shard_{rep}")
            h1_full = drp.tile([N, HID], L2DT, name=f"h1_full_{rep}",
                               addr_space="Shared")
            p_shard = drp.tile([S, OUTP], F32, name=f"p_shard_{rep}")
            p_full = drp.tile([N, OUTP], F32, name=f"p_full_{rep}",
                              addr_space="Shared")
            st_in = [drp.tile([P, 4], F32, name=f"st_in{l}_{rep}")
                     for l in range(2)]
            st_out = [drp.tile([P, 4], F32, name=f"st_out{l}_{rep}")
                      for l in range(2)]
            # ================= Layer 1 =================
            xT_sb = bigp.tile([P, S], F32, tag="big", name="xT_sb")
            nc.sync.dma_start(xT_sb[:], xT[:, :])
            x_lo = xfull[:, :]
            x_hi = xfull[HALF:N, :]
            zT1, szp1, ssqp1 = dense_layer("L1", INDIM, x_lo, x_hi, [xT_sb],
                                           [w1l_sb], [w1r_sb], tdt=TDT)
            h1T = bn_relu(0, "L1", zT1, szp1, ssqp1, g1_sb, be1_sb)

            # rebuild node-major h1 and AllGather the full table
            for wi in range(NW):
                rows = P if wi < NW - 1 else LASTW
                hnm = agp.tile([P, HID], L2DT, tag="agg", name=f"hnm_{wi}")
                for h in range(2):
                    pst = ptr.tile([P, P], F32, tag="tr", name=f"ptrh_{wi}_{h}")
                    nc.tensor.transpose(pst[:rows, :],
                                        h1T[h][:, wi * P : wi * P + rows],
                                        ident[:])
                    nc.vector.tensor_copy(out=hnm[:rows, h * P : (h + 1) * P],
                                          in_=pst[:rows, :])
                nc.sync.dma_start(h1_shard[wi * P : wi * P + rows, :],
                                  hnm[:rows, :])
            collective("AllGather", AL.bypass, [h1_shard[:, :]], [h1_full[:, :]])

            # ================= Layer 2 =================
            h1_lo = h1_full[:, :]
            h1_hi = h1_full[HALF:N, :]
            zT2, szp2, ssqp2 = dense_layer("L2", HID, h1_lo, h1_hi, h1T,
                                           w2l_sb, w2r_sb, tdt=L2DT)
            h2T = bn_relu(1, "L2", zT2, szp2, ssqp2, g2_sb, be2_sb)

            # ================= Layer 3 =================
            # p = h2 @ w3l (padded to 64 cols), rebuilt node-major + AllGather
            pT = bigp.tile([P, S], F32, tag="big", name="pT")  # only rows :64 used
            for b in range(NBLK):
                n0 = b * BLK
                nb = min(S, n0 + BLK) - n0
                psp = pz.tile([P, BLK], F32, tag="z", name=f"pzp_{b}")
                for h in range(2):
                    nc.tensor.matmul(
                        out=psp[:OUTP, :nb], lhsT=_mm_dt(w3l_sb[h][:]),
                        rhs=_mm_dt(h2T[h][:, n0 : n0 + nb]),
                        start=(h == 0), stop=(h == 1),
                    )
                nc.vector.tensor_copy(out=pT[:OUTP, n0 : n0 + nb],
                                      in_=psp[:OUTP, :nb])
            for wi in range(NW):
                rows = P if wi < NW - 1 else LASTW
                pnm = agp.tile([P, OUTP], F32, tag="agg", name=f"pnm_{wi}")
                pst = ptr.tile([P, P], F32, tag="tr", name=f"ptrp_{wi}")
                nc.tensor.transpose(pst[:rows, :OUTP],
                                    pT[:OUTP, wi * P : wi * P + rows],
                                    ident[:OUTP, :OUTP])
                nc.vector.tensor_copy(out=pnm[:rows, :], in_=pst[:rows, :OUTP])
                nc.sync.dma_start(p_shard[wi * P : wi * P + rows, :], pnm[:rows, :])
            collective("AllGather", AL.bypass, [p_shard[:, :]], [p_full[:, :]])

            # z3r^T = w3r^T @ h2^T + b3 (feature-major, 47 rows)
            z3rT = bigp.tile([P, S], F32, tag="big", name="z3rT")
            for b in range(NBLK):
                n0 = b * BLK
                nb = min(S, n0 + BLK) - n0
                psr = pz.tile([P, BLK], F32, tag="z", name=f"pzr_{b}")
                for h in range(2):
                    nc.tensor.matmul(
                        out=psr[:OUT, :nb], lhsT=_mm_dt(w3r_sb[h][:]),
                        rhs=_mm_dt(h2T[h][:, n0 : n0 + nb]),
                        start=(h == 0), stop=(h == 1),
                    )
                nc.vector.tensor_scalar(out=z3rT[:OUT, n0 : n0 + nb],
                                        in0=psr[:OUT, :nb],
                                        scalar1=b3_sb[:OUT, :], scalar2=None,
                                        op0=AL.add)

            # aggregate p, combine, log_softmax, store
            out_sb = bigp.tile([P, NW * OUT], F32, tag="outsb", bufs=1,
                               name="out_sb")
            p_lo = p_full[:, :]
            p_hi = p_full[HALF:N, :]
            for wi in range(NW):
                rows = P if wi < NW - 1 else LASTW
                aggw = gather_segsum_window(wi, OUTP, p_lo, p_hi, "L3")
                pst = ptr.tile([P, P], F32, tag="tr", name=f"ptrz_{wi}")
                nc.tensor.transpose(pst[:rows, :OUT],
                                    z3rT[:OUT, wi * P : wi * P + rows],
                                    ident[:OUT, :OUT])
                z3w = agp.tile([P, OUT], F32, tag="agg", name=f"z3w_{wi}")
                nc.vector.tensor_tensor(out=z3w[:rows, :], in0=aggw[:rows, :OUT],
                                        in1=pst[:rows, :OUT], op=AL.add)
                negmax = smp.tile([P, 1], F32, tag="negmax", name=f"negmax_{wi}")
                nc.vector.tensor_reduce(out=negmax[:rows, :], in_=z3w[:rows, :],
                                        axis=mybir.AxisListType.X, op=AL.max,
                                        negate=True)
                esc = smp.tile([P, OUT], F32, tag="esc", name=f"esc_{wi}")
                sume = smp.tile([P, 1], F32, tag="sume", name=f"sume_{wi}")
                nc.scalar.activation(out=esc[:rows, :], in_=z3w[:rows, :],
                                     func=AF.Exp, bias=negmax[:rows, :], scale=1.0,
                                     accum_out=sume[:rows, :])
                logsum = smp.tile([P, 1], F32, tag="logsum", name=f"logsum_{wi}")
                nc.scalar.activation(out=logsum[:rows, :], in_=sume[:rows, :],
                                     func=AF.Ln)
                nc.vector.tensor_scalar(
                    out=out_sb[:rows, wi * OUT : (wi + 1) * OUT],
                    in0=z3w[:rows, :],
                    scalar1=negmax[:rows, :], scalar2=logsum[:rows, :],
                    op0=AL.add, op1=AL.subtract,
                )
            # store (full windows in one strided DMA, tail window separately)
            nfull = NW - 1
            nc.sync.dma_start(
                out_d[0 : nfull * P, :].rearrange("(w p) f -> p w f", p=P),
                out_sb[:].rearrange("p (w f) -> p w f", f=OUT)[:, :nfull, :],
            )
            nc.sync.dma_start(
                out_d[nfull * P : S, :],
                out_sb[:LASTW, nfull * OUT : NW * OUT],
            )

    nc.compile()
    return nc


# --------------------------------------------------------------------------
# Entry point
# --------------------------------------------------------------------------

def _make_in_maps(inputs, meta, idx_arrs, dslot_arrs, invd_arrs):
    N = meta["N"]
    S = meta["S"]
    x = np.ascontiguousarray(np.asarray(inputs["x"], dtype=np.float32))
    xg = x.astype(np.float16) if GATHER_F16 else x
    OUT = np.asarray(inputs["b3"]).shape[0]
    HID = np.asarray(inputs["b1"]).shape[0]

    def bn_pack(v):
        return np.ascontiguousarray(
            np.asarray(v, dtype=np.float32).reshape(2, P).T
        )

    w3l_pad = np.zeros((HID, 64), dtype=np.float32)
    w3l_pad[:, :OUT] = np.asarray(inputs["w3l"], dtype=np.float32)
    b3_pad = np.zeros((P, 1), dtype=np.float32)
    b3_pad[:OUT, 0] = np.asarray(inputs["b3"], dtype=np.float32)

    shared = dict(
        xfull=xg,
        w1l=np.ascontiguousarray(np.asarray(inputs["w1l"], np.float32)),
        w1r=np.ascontiguousarray(np.asarray(inputs["w1r"], np.float32)),
        w2l=np.ascontiguousarray(np.asarray(inputs["w2l"], np.float32)),
        w2r=np.ascontiguousarray(np.asarray(inputs["w2r"], np.float32)),
        w3l=w3l_pad,
        w3r=np.ascontiguousarray(np.asarray(inputs["w3r"], np.float32)),
        g1=bn_pack(inputs["g1"]), be1=bn_pack(inputs["be1"]),
        g2=bn_pack(inputs["g2"]), be2=bn_pack(inputs["be2"]),
        b3=b3_pad,
    )
    in_maps = []
    for c in range(NCORES):
        m = dict(shared)
        m["xT"] = np.ascontiguousarray(x[c * S : (c + 1) * S, :].T)
        m["idx"] = idx_arrs[c]
        m["dslot"] = dslot_arrs[c]
        m["invd"] = invd_arrs[c]
        in_maps.append(m)
    return in_maps


_CACHE = {}


def _get_compiled(inputs):
    N, INDIM = np.asarray(inputs["x"]).shape
    HID = np.asarray(inputs["b1"]).shape[0]
    OUT = np.asarray(inputs["b3"]).shape[0]
    ei = np.ascontiguousarray(np.asarray(inputs["edge_index"], dtype=np.int64))
    key = (N, INDIM, HID, OUT, hash(ei.tobytes()))
    meta, idx_arrs, dslot_arrs, invd_arrs = preprocess(ei, N)
    if key not in _CACHE:
        _CACHE[key] = build_program(meta, INDIM, HID, OUT)
    return _CACHE[key], meta, idx_arrs, dslot_arrs, invd_arrs


def kernel(**inputs):
    nc, meta, idx_arrs, dslot_arrs, invd_arrs = _get_compiled(inputs)
    in_maps = _make_in_maps(inputs, meta, idx_arrs, dslot_arrs, invd_arrs)
    res = run_bass_kernel_spmd(nc, in_maps, core_ids=list(range(NCORES)))
    return np.concatenate([r["out"] for r in res.results], axis=0)





# revision 2
# speedup vs baseline: 2.5711x; 2.5711x over previous
"""ExpanderSAGE GNN kernel for 8x Trainium2 NeuronCores (Bass/Tile).

Strategy (graph/data parallel, dst-sharded):
  - 50000 nodes sharded 6250/core (8 cores). Each core owns the edges whose
    dst lands in its shard, sorted into (block, half, window) groups where a
    block = 4 consecutive 128-dst windows (512 nodes) and half = src < N/2
    (dma_gather indices are int16, so the node table is split in two halves).
  - Neighbor aggregation: ONE dma_gather per (block, half) fetches all of the
    block's src rows (~4.6k edges) from the node-major table in HBM — few
    SWDGE calls, each with ~300 descriptors (16 idx/descriptor).  A one-hot
    (edge -> dst slot) matrix built with iota+is_equal on DVE turns the
    per-128-edge chunks into PSUM matmul accumulations per window.
  - All dense math in f16 (weights, activations, one-hot, gather tables);
    PSUM accumulation stays f32.  inv_deg scaling fused into PSUM evacuation.
  - BatchNorm: per-core sum / sum-of-squares partials -> AllReduce(8 cores)
    -> affine+ReLU applied via DVE + ACT.  Layer-1 output is rebuilt
    node-major (f16) and AllGather'd to form the next gather table.  Layer 3
    pre-projects p = h2 @ w3l (47 -> pad 128 f16 cols = 256B rows) so only
    256B rows are gathered; log_softmax is computed node-major per window.
"""

import os
import sys

import numpy as np

for _p in ("/opt/trn_rl_repo", os.path.expanduser("~/.axon_site/_ro/trn_rl_repo")):
    if os.path.isdir(_p) and _p not in sys.path:
        sys.path.insert(0, _p)

import concourse.bacc as bacc
import concourse.mybir as mybir
import concourse.tile as tile
from concourse.bass_utils import run_bass_kernel_spmd
from concourse.masks import make_identity

F32 = mybir.dt.float32
F16 = mybir.dt.float16
F8 = mybir.dt.float8e4
I16 = mybir.dt.int16
I32 = mybir.dt.int32
AL = mybir.AluOpType
AF = mybir.ActivationFunctionType

EPS = 1e-5
NCORES = 8
P = 128
BLK = 512  # node block for dense matmuls (= 4 windows)
WPB = BLK // P

# fp8 e4m3 for the layer-2 neighbor table (halves the dominant gather term;
# the self term and all dense math stay f16)
GATHER_F8L2 = os.environ.get("GATHER_F8L2", "0") == "1"
# max indices per dma_gather call (multiple of 128)
GCALL = int(os.environ.get("GCALL", "4608"))


# --------------------------------------------------------------------------
# Host-side preprocessing: shard edges, build gather-index / one-hot inputs
# --------------------------------------------------------------------------

def preprocess(edge_index, n_nodes):
    src = np.asarray(edge_index[0], dtype=np.int64)
    dst = np.asarray(edge_index[1], dtype=np.int64)
    S = n_nodes // NCORES
    NW = (S + P - 1) // P
    NB = (NW + WPB - 1) // WPB
    HALF = n_nodes // 2
    assert HALF < 32768 and n_nodes - HALF < 32768

    deg = np.bincount(dst, minlength=n_nodes).astype(np.float32)
    invdeg = (1.0 / np.maximum(deg, 1.0)).astype(np.float32)

    core = dst // S
    local = dst - core * S
    w = local // P
    slot = local % P
    blk = w // WPB
    half = (src >= HALF).astype(np.int64)
    idxval = (src - half * HALF).astype(np.int64)

    # group key (core, block, half, window); stable sort groups the edges
    wib = w - blk * WPB
    key = ((core * NB + blk) * 2 + half) * WPB + wib
    order = np.argsort(key, kind="stable")
    skey = key[order]
    sidx = idxval[order]
    sslot = slot[order]

    ngroups = NCORES * NB * 2 * WPB
    counts = np.bincount(skey, minlength=ngroups).reshape(NCORES, NB, 2, WPB)
    starts = np.zeros(ngroups + 1, dtype=np.int64)
    np.cumsum(counts.reshape(-1), out=starts[1:])

    # uniform chunk counts across cores (SPMD: one NEFF for all 8 cores)
    C = np.ceil(counts.max(axis=0) / P).astype(np.int64)  # [NB, 2, WPB]
    CTOT = int(C.sum())
    LTOT = CTOT * P

    # per-(b,h,w) chunk-column offsets into the concatenated device arrays
    chunk_off = np.zeros((NB, 2, WPB), dtype=np.int64)
    acc = 0
    for b in range(NB):
        for h in range(2):
            for wi in range(WPB):
                chunk_off[b, h, wi] = acc
                acc += C[b, h, wi]

    idx_arrs = []
    dslot_arrs = []
    for c in range(NCORES):
        idx_a = np.zeros((16, LTOT // 16), dtype=np.int16)
        ds_a = np.full((P, CTOT), -1.0, dtype=np.float32)
        for b in range(NB):
            for h in range(2):
                for wi in range(WPB):
                    cg = int(C[b, h, wi])
                    if cg == 0:
                        continue
                    g = ((c * NB + b) * 2 + h) * WPB + wi
                    s0, s1 = starts[g], starts[g + 1]
                    k = s1 - s0
                    Lw = cg * P
                    buf = np.zeros(Lw, dtype=np.int16)
                    buf[:k] = sidx[s0:s1].astype(np.int16)
                    co = int(chunk_off[b, h, wi])
                    idx_a[:, co * 8 : co * 8 + Lw // 16] = buf.reshape(-1, 16).T
                    sl = np.full(Lw, -1.0, dtype=np.float32)
                    sl[:k] = sslot[s0:s1].astype(np.float32)
                    ds_a[:, co : co + cg] = sl.reshape(cg, P).T
        idx_arrs.append(np.tile(idx_a, (8, 1)))  # replicate to 128 partitions
        dslot_arrs.append(ds_a)

    # invdeg per core, [128, NW] (partition = slot, col = window), pad 1.0
    invd_arrs = []
    for c in range(NCORES):
        v = np.ones(NW * P, dtype=np.float32)
        v[:S] = invdeg[c * S : (c + 1) * S]
        invd_arrs.append(v.reshape(NW, P).T.copy())

    meta = dict(N=n_nodes, S=S, NW=NW, NB=NB, HALF=HALF, C=C,
                chunk_off=chunk_off, CTOT=CTOT, ITOT=LTOT // 16)
    return meta, idx_arrs, dslot_arrs, invd_arrs


# --------------------------------------------------------------------------
# Device program
# --------------------------------------------------------------------------

def build_program(meta, INDIM, HID, OUT, reps=1, ncores=NCORES, mock_cc=False,
                  abl=()):
    N, S, NW, NB, HALF = (meta["N"], meta["S"], meta["NW"], meta["NB"],
                          meta["HALF"])
    C, chunk_off, CTOT, ITOT = (meta["C"], meta["chunk_off"], meta["CTOT"],
                                meta["ITOT"])
    LASTW = S - P * (NW - 1)
    NBLK = (S + BLK - 1) // BLK
    assert NBLK == NB
    OUTP = 128  # padded projection width for layer 3 (47 -> 128, f16 = 256B)
    RG = [list(range(ncores))]

    nc = bacc.Bacc("TRN2", target_bir_lowering=False, debug=False,
                   num_devices=ncores, num_swdge_queues=4)
    qctr = [0]

    # ---- I/O ----
    L2DT = F8 if GATHER_F8L2 else F16
    xfull = nc.dram_tensor("xfull", [N, INDIM], F16, kind="ExternalInput")
    xT = nc.dram_tensor("xT", [P, S], F16, kind="ExternalInput")
    idx_d = nc.dram_tensor("idx", [P, ITOT], I16, kind="ExternalInput")
    dslot_d = nc.dram_tensor("dslot", [P, CTOT], F32, kind="ExternalInput")
    invd_d = nc.dram_tensor("invd", [P, NW], F32, kind="ExternalInput")
    w1l_d = nc.dram_tensor("w1l", [INDIM, HID], F16, kind="ExternalInput")
    w1r_d = nc.dram_tensor("w1r", [INDIM, HID], F16, kind="ExternalInput")
    w2l_d = nc.dram_tensor("w2l", [HID, HID], F16, kind="ExternalInput")
    w2r_d = nc.dram_tensor("w2r", [HID, HID], F16, kind="ExternalInput")
    w3l_d = nc.dram_tensor("w3l", [HID, OUTP], F16, kind="ExternalInput")
    w3r_d = nc.dram_tensor("w3r", [HID, OUT], F16, kind="ExternalInput")
    g1_d = nc.dram_tensor("g1", [P, 2], F32, kind="ExternalInput")
    be1_d = nc.dram_tensor("be1", [P, 2], F32, kind="ExternalInput")
    g2_d = nc.dram_tensor("g2", [P, 2], F32, kind="ExternalInput")
    be2_d = nc.dram_tensor("be2", [P, 2], F32, kind="ExternalInput")
    b3_d = nc.dram_tensor("b3", [P, 1], F32, kind="ExternalInput")
    out_d = nc.dram_tensor("out", [S, OUT], F32, kind="ExternalOutput")

    from contextlib import ExitStack

    with tile.TileContext(nc) as tc, ExitStack() as es:
        cp = es.enter_context(tc.tile_pool(name="const", bufs=1))
        gp = es.enter_context(tc.tile_pool(name="gath", bufs=3))
        ohp = es.enter_context(tc.tile_pool(name="oh", bufs=3))
        agp = es.enter_context(tc.tile_pool(name="agg", bufs=4))
        atp = es.enter_context(tc.tile_pool(name="aggT", bufs=4))
        bigp = es.enter_context(tc.tile_pool(name="big", bufs=4))
        sqp = es.enter_context(tc.tile_pool(name="sq", bufs=2))
        smp = es.enter_context(tc.tile_pool(name="small", bufs=2))
        pseg = es.enter_context(tc.tile_pool(name="pseg", bufs=4, space="PSUM"))
        ptr = es.enter_context(tc.tile_pool(name="ptr", bufs=2, space="PSUM"))
        pz = es.enter_context(tc.tile_pool(name="pz", bufs=2, space="PSUM"))
        drp = es.enter_context(tc.tile_pool(name="dram", bufs=1, space="DRAM"))

        # ---- constants ----
        idx_sb = cp.tile([P, ITOT], I16, name="idx_sb")
        nc.sync.dma_start(idx_sb[:], idx_d[:, :])
        dslot_sb = cp.tile([P, CTOT], F32, name="dslot_sb")
        nc.sync.dma_start(dslot_sb[:], dslot_d[:, :])
        invd_sb = cp.tile([P, NW], F32, name="invd_sb")
        nc.sync.dma_start(invd_sb[:], invd_d[:, :])

        w1l_sb = cp.tile([P, HID], F16, name="w1l_sb")
        nc.sync.dma_start(w1l_sb[:], w1l_d[:, :])
        w1r_sb = cp.tile([P, HID], F16, name="w1r_sb")
        nc.sync.dma_start(w1r_sb[:], w1r_d[:, :])
        w2l_sb = [cp.tile([P, HID], F16, name=f"w2l_sb{k}") for k in range(2)]
        w2r_sb = [cp.tile([P, HID], F16, name=f"w2r_sb{k}") for k in range(2)]
        w3l_sb = [cp.tile([P, OUTP], F16, name=f"w3l_sb{k}") for k in range(2)]
        w3r_sb = [cp.tile([P, OUT], F16, name=f"w3r_sb{k}") for k in range(2)]
        for k in range(2):
            nc.sync.dma_start(w2l_sb[k][:], w2l_d[k * P : (k + 1) * P, :])
            nc.sync.dma_start(w2r_sb[k][:], w2r_d[k * P : (k + 1) * P, :])
            nc.sync.dma_start(w3l_sb[k][:], w3l_d[k * P : (k + 1) * P, :])
            nc.sync.dma_start(w3r_sb[k][:], w3r_d[k * P : (k + 1) * P, :])
        g1_sb = cp.tile([P, 2], F32, name="g1_sb")
        nc.sync.dma_start(g1_sb[:], g1_d[:, :])
        be1_sb = cp.tile([P, 2], F32, name="be1_sb")
        nc.sync.dma_start(be1_sb[:], be1_d[:, :])
        g2_sb = cp.tile([P, 2], F32, name="g2_sb")
        nc.sync.dma_start(g2_sb[:], g2_d[:, :])
        be2_sb = cp.tile([P, 2], F32, name="be2_sb")
        nc.sync.dma_start(be2_sb[:], be2_d[:, :])
        b3_sb = cp.tile([P, 1], F32, name="b3_sb")
        nc.sync.dma_start(b3_sb[:], b3_d[:, :])

        iota_i = cp.tile([P, P], I32, name="iota_i")
        nc.gpsimd.iota(iota_i[:], pattern=[[1, P]], base=0, channel_multiplier=0)
        iota_h = cp.tile([P, P], F16, name="iota_h")
        nc.vector.tensor_copy(iota_h[:], iota_i[:])
        dslot_h = cp.tile([P, CTOT], F16, name="dslot_h")
        nc.vector.tensor_copy(dslot_h[:], dslot_sb[:])
        ident_h = cp.tile([P, P], F16, name="ident_h")
        make_identity(nc, ident_h[:])
        eps_sb = cp.tile([P, 1], F32, name="eps_sb")
        nc.vector.memset(eps_sb[:], EPS)

        if GATHER_F8L2:
            iota_8 = cp.tile([P, P], F8, name="iota_8")
            nc.vector.tensor_copy(iota_8[:], iota_i[:])
            dslot_8 = cp.tile([P, CTOT], F8, name="dslot_8")
            nc.vector.tensor_copy(dslot_8[:], dslot_sb[:])

        def gather_block(b, F, lo_ap, hi_ap, lname, tdt):
            """One dma_gather + one-hot build per (block, half). Returns
            (gv, ohv) lists indexed by half, each [128, CB_h, F/P-dims]."""
            gvs, ohvs = [], []
            for h, base_ap in ((0, lo_ap), (1, hi_ap)):
                cb = int(C[b, h, :].sum())
                if cb == 0:
                    gvs.append(None)
                    ohvs.append(None)
                    continue
                co = int(chunk_off[b, h, 0])
                g_t = gp.tile([P, cb * F], tdt, tag="g", name=f"g{lname}_{b}_{h}")
                gv = g_t[:].rearrange("p (c f) -> p c f", f=F)
                maxc = max(1, GCALL // P)
                npieces = -(-cb // maxc)
                sizes = [cb // npieces + (1 if i < cb % npieces else 0)
                         for i in range(npieces)]
                offs = [sum(sizes[:i]) for i in range(npieces)]
                if "nogather" in abl:
                    nc.vector.memset(g_t[:, 0:1], 0.0)
                else:
                    for c0, cn in zip(offs, sizes):
                        nc.gpsimd.dma_gather(
                            out_ap=gv[:, c0 : c0 + cn, :],
                            in_ap=base_ap,
                            idxs_ap=idx_sb[:, (co + c0) * 8 : (co + c0 + cn) * 8],
                            num_idxs=cn * P,
                            num_idxs_reg=cn * P,
                            elem_size=F,
                            queue_num=qctr[0] % 4,
                        )
                        qctr[0] += 1
                oh_t = ohp.tile([P, cb * P], tdt, tag="oh",
                                name=f"oh{lname}_{b}_{h}")
                ohv = oh_t[:].rearrange("p (c q) -> p c q", q=P)
                if "noonehot" in abl:
                    nc.vector.memset(oh_t[:, 0:1], 0.0)
                else:
                    iota_t = iota_8 if tdt == F8 else iota_h
                    dslot_t = dslot_8 if tdt == F8 else dslot_h
                    nc.vector.tensor_tensor(
                        out=ohv,
                        in0=iota_t[:].unsqueeze(1).to_broadcast([P, cb, P]),
                        in1=dslot_t[:, co : co + cb].unsqueeze(2).to_broadcast(
                            [P, cb, P]
                        ),
                        op=AL.is_equal,
                    )
                gvs.append(gv)
                ohvs.append(ohv)
            return gvs, ohvs

        def segsum_window(b, wi, F, gvs, ohvs, lname):
            """Accumulate window wi of block b from the block's gathered
            tiles; returns scaled agg tile [128, F] f16 (partition=dst)."""
            wg = b * WPB + wi
            ctot = int(C[b, :, wi].sum())
            aggw = agp.tile([P, F], F16, tag="agg", name=f"agg{lname}_{wg}")
            if ctot == 0:
                nc.vector.memset(aggw[:], 0.0)
                return aggw
            ps = pseg.tile([P, F], F32, tag="seg", name=f"pseg{lname}_{wg}")
            done = 0
            for h in range(2):
                cg = int(C[b, h, wi])
                if cg == 0:
                    continue
                lo = int(chunk_off[b, h, wi] - chunk_off[b, h, 0])
                for ch in range(lo, lo + cg):
                    if "nosegmm" in abl:
                        done += 1
                        continue
                    nc.tensor.matmul(
                        out=ps[:],
                        lhsT=ohvs[h][:, ch, :],
                        rhs=gvs[h][:, ch, :],
                        start=(done == 0),
                        stop=(done == ctot - 1),
                    )
                    done += 1
            nc.vector.tensor_scalar(
                out=aggw[:], in0=ps[:], scalar1=invd_sb[:, wg : wg + 1],
                scalar2=None, op0=AL.mult,
            )
            return aggw

        def dense_layer(lname, F_in, lo_ap, hi_ap, hT, wl_sb, wr_sb, tdt):
            """Full SAGE layer (aggregate + dense), feature-major output.
            Returns (zT halves f16, sum partials, sumsq partials)."""
            nh_in = F_in // P
            zT = [bigp.tile([P, S], F16, tag="big", name=f"zT{lname}_{m}")
                  for m in range(2)]
            szp = [smp.tile([P, NBLK], F32, tag=f"szp{lname}{m}",
                            name=f"szp{lname}{m}") for m in range(2)]
            ssqp = [smp.tile([P, NBLK], F32, tag=f"ssqp{lname}{m}",
                             name=f"ssqp{lname}{m}") for m in range(2)]
            for b in range(NBLK):
                n0 = b * BLK
                nb = min(S, n0 + BLK) - n0
                nw_b = min(WPB, NW - b * WPB)
                gvs, ohvs = gather_block(b, F_in, lo_ap, hi_ap, lname, tdt)
                aggT = [atp.tile([P, BLK], F16, tag="aggT",
                                 name=f"aggT{lname}_{b}_{h}")
                        for h in range(nh_in)]
                for wi in range(nw_b):
                    aggw = segsum_window(b, wi, F_in, gvs, ohvs, lname)
                    for h in range(nh_in):
                        pst = ptr.tile([P, P], F32, tag="tr",
                                       name=f"ptr{lname}_{b}_{wi}_{h}")
                        nc.tensor.transpose(
                            pst[:], aggw[:, h * P : (h + 1) * P], ident_h[:]
                        )
                        c0 = wi * P
                        cw = min(P, nb - c0)
                        nc.vector.tensor_copy(
                            out=aggT[h][:, c0 : c0 + cw], in_=pst[:, :cw]
                        )
                for m in range(2):
                    psz = pz.tile([P, BLK], F32, tag="z",
                                  name=f"pz{lname}_{b}_{m}")
                    mcols = slice(m * P, (m + 1) * P)
                    nmm = 2 * nh_in
                    k = 0
                    for h in range(nh_in):
                        nc.tensor.matmul(
                            out=psz[:, :nb],
                            lhsT=wl_sb[h][:, mcols],
                            rhs=aggT[h][:, :nb],
                            start=(k == 0), stop=(k == nmm - 1),
                        )
                        k += 1
                    for h in range(nh_in):
                        nc.tensor.matmul(
                            out=psz[:, :nb],
                            lhsT=wr_sb[h][:, mcols],
                            rhs=hT[h][:, n0 : n0 + nb],
                            start=(k == 0), stop=(k == nmm - 1),
                        )
                        k += 1
                    nc.vector.tensor_reduce(
                        out=szp[m][:, b : b + 1], in_=psz[:, :nb],
                        axis=mybir.AxisListType.X, op=AL.add,
                    )
                    sqsc = sqp.tile([P, BLK], F32, tag="sq",
                                    name=f"sq{lname}_{b}_{m}")
                    nc.scalar.activation(
                        out=sqsc[:, :nb], in_=psz[:, :nb], func=AF.Square,
                        accum_out=ssqp[m][:, b : b + 1],
                    )
                    nc.vector.tensor_copy(out=zT[m][:, n0 : n0 + nb],
                                          in_=psz[:, :nb])
            return zT, szp, ssqp

        def collective(kind, op, ins, outs):
            if mock_cc:
                nc.sync.dma_start(outs[0][0 : ins[0].shape[0]], ins[0])
            else:
                nc.gpsimd.collective_compute(kind, op, replica_groups=RG,
                                             ins=ins, outs=outs)

        def bn_relu(li, lname, zT, szp, ssqp, g_sb, be_sb):
            """AllReduce stats, then hT = relu((z - mean) * a + be), f16."""
            stat = smp.tile([P, 4], F32, tag=f"stat{lname}", name=f"stat{lname}")
            for m in range(2):
                nc.vector.tensor_reduce(out=stat[:, m : m + 1], in_=szp[m][:],
                                        axis=mybir.AxisListType.X, op=AL.add)
                nc.vector.tensor_reduce(out=stat[:, 2 + m : 3 + m],
                                        in_=ssqp[m][:],
                                        axis=mybir.AxisListType.X, op=AL.add)
            nc.sync.dma_start(st_in[li][:], stat[:])
            collective("AllReduce", AL.add, [st_in[li][:]], [st_out[li][:]])
            statg = smp.tile([P, 4], F32, tag=f"statg{lname}", name=f"statg{lname}")
            nc.sync.dma_start(statg[:], st_out[li][:])
            hT = []
            for m in range(2):
                mean = smp.tile([P, 1], F32, tag=f"mean{lname}{m}",
                                name=f"mean{lname}{m}")
                nc.vector.tensor_scalar(out=mean[:], in0=statg[:, m : m + 1],
                                        scalar1=1.0 / N, scalar2=None,
                                        op0=AL.mult)
                ex2 = smp.tile([P, 1], F32, tag=f"ex2{lname}{m}",
                               name=f"ex2{lname}{m}")
                nc.vector.tensor_scalar(out=ex2[:], in0=statg[:, 2 + m : 3 + m],
                                        scalar1=1.0 / N, scalar2=None,
                                        op0=AL.mult)
                nvar = smp.tile([P, 1], F32, tag=f"nvar{lname}{m}",
                                name=f"nvar{lname}{m}")
                # nvar = mean^2 - E[x^2]  (= -var)
                nc.vector.scalar_tensor_tensor(
                    out=nvar[:], in0=mean[:], scalar=mean[:], in1=ex2[:],
                    op0=AL.mult, op1=AL.subtract,
                )
                std = smp.tile([P, 1], F32, tag=f"std{lname}{m}",
                               name=f"std{lname}{m}")
                nc.scalar.activation(out=std[:], in_=nvar[:], func=AF.Sqrt,
                                     bias=eps_sb[:], scale=-1.0)
                istd = smp.tile([P, 1], F32, tag=f"istd{lname}{m}",
                                name=f"istd{lname}{m}")
                nc.vector.reciprocal(istd[:], std[:])
                a_m = smp.tile([P, 1], F32, tag=f"a{lname}{m}",
                               name=f"a{lname}{m}")
                nc.vector.tensor_tensor(out=a_m[:], in0=g_sb[:, m : m + 1],
                                        in1=istd[:], op=AL.mult)
                h_m = bigp.tile([P, S], F16, tag="big", name=f"hT{lname}_{m}")
                nc.vector.tensor_scalar(out=h_m[:], in0=zT[m][:],
                                        scalar1=mean[:], scalar2=a_m[:],
                                        op0=AL.subtract, op1=AL.mult)
                nc.scalar.activation(out=h_m[:], in_=h_m[:], func=AF.Relu,
                                     bias=be_sb[:, m : m + 1], scale=1.0)
                hT.append(h_m)
            return hT

        for rep in range(reps):
            # ---- DRAM intermediates (fresh per rep: Shared tiles are
            # single-writer) ----
            h1_shard = drp.tile([S, HID], L2DT, name=f"h1_shard_{rep}")
            h1_full = drp.tile([N, HID], L2DT, name=f"h1_full_{rep}",
                               addr_space="Shared")
            p_shard = drp.tile([S, OUTP], F16, name=f"p_shard_{rep}")
            p_full = drp.tile([N, OUTP], F16, name=f"p_full_{rep}",
                              addr_space="Shared")
            st_in = [drp.tile([P, 4], F32, name=f"st_in{l}_{rep}")
                     for l in range(2)]
            st_out = [drp.tile([P, 4], F32, name=f"st_out{l}_{rep}")
                      for l in range(2)]
            # ================= Layer 1 =================
            xT_sb = bigp.tile([P, S], F16, tag="big", name="xT_sb")
            nc.sync.dma_start(xT_sb[:], xT[:, :])
            x_lo = xfull[:, :]
            x_hi = xfull[HALF:N, :]
            zT1, szp1, ssqp1 = dense_layer("L1", INDIM, x_lo, x_hi, [xT_sb],
                                           [w1l_sb], [w1r_sb], tdt=F16)
            h1T = bn_relu(0, "L1", zT1, szp1, ssqp1, g1_sb, be1_sb)

            # rebuild node-major h1 and AllGather the full table
            for wi in range(NW):
                rows = P if wi < NW - 1 else LASTW
                hnm = agp.tile([P, HID], L2DT, tag="agg", name=f"hnm_{wi}")
                for h in range(2):
                    pst = ptr.tile([P, P], F32, tag="tr", name=f"ptrh_{wi}_{h}")
                    nc.tensor.transpose(pst[:rows, :],
                                        h1T[h][:, wi * P : wi * P + rows],
                                        ident_h[:])
                    nc.vector.tensor_copy(out=hnm[:rows, h * P : (h + 1) * P],
                                          in_=pst[:rows, :])
                nc.sync.dma_start(h1_shard[wi * P : wi * P + rows, :],
                                  hnm[:rows, :])
            collective("AllGather", AL.bypass, [h1_shard[:, :]], [h1_full[:, :]])

            # ================= Layer 2 =================
            h1_lo = h1_full[:, :]
            h1_hi = h1_full[HALF:N, :]
            zT2, szp2, ssqp2 = dense_layer("L2", HID, h1_lo, h1_hi, h1T,
                                           w2l_sb, w2r_sb, tdt=L2DT)
            h2T = bn_relu(1, "L2", zT2, szp2, ssqp2, g2_sb, be2_sb)

            # ================= Layer 3 =================
            # p = h2 @ w3l (padded to 128 f16 cols), rebuilt node-major + AG
            pT = bigp.tile([P, S], F16, tag="big", name="pT")
            for b in range(NBLK):
                n0 = b * BLK
                nb = min(S, n0 + BLK) - n0
                psp = pz.tile([P, BLK], F32, tag="z", name=f"pzp_{b}")
                for h in range(2):
                    nc.tensor.matmul(
                        out=psp[:OUTP, :nb], lhsT=w3l_sb[h][:],
                        rhs=h2T[h][:, n0 : n0 + nb],
                        start=(h == 0), stop=(h == 1),
                    )
                nc.vector.tensor_copy(out=pT[:OUTP, n0 : n0 + nb],
                                      in_=psp[:OUTP, :nb])
            for wi in range(NW):
                rows = P if wi < NW - 1 else LASTW
                pnm = agp.tile([P, OUTP], F16, tag="agg", name=f"pnm_{wi}")
                pst = ptr.tile([P, P], F32, tag="tr", name=f"ptrp_{wi}")
                nc.tensor.transpose(pst[:rows, :OUTP],
                                    pT[:OUTP, wi * P : wi * P + rows],
                                    ident_h[:OUTP, :OUTP])
                nc.vector.tensor_copy(out=pnm[:rows, :], in_=pst[:rows, :OUTP])
                nc.sync.dma_start(p_shard[wi * P : wi * P + rows, :], pnm[:rows, :])
            collective("AllGather", AL.bypass, [p_shard[:, :]], [p_full[:, :]])

            # z3r^T = w3r^T @ h2^T + b3 (feature-major, 47 rows)
            z3rT = bigp.tile([P, S], F32, tag="bigz", bufs=1, name="z3rT")
            for b in range(NBLK):
                n0 = b * BLK
                nb = min(S, n0 + BLK) - n0
                psr = pz.tile([P, BLK], F32, tag="z", name=f"pzr_{b}")
                for h in range(2):
                    nc.tensor.matmul(
                        out=psr[:OUT, :nb], lhsT=w3r_sb[h][:],
                        rhs=h2T[h][:, n0 : n0 + nb],
                        start=(h == 0), stop=(h == 1),
                    )
                nc.vector.tensor_scalar(out=z3rT[:OUT, n0 : n0 + nb],
                                        in0=psr[:OUT, :nb],
                                        scalar1=b3_sb[:OUT, :], scalar2=None,
                                        op0=AL.add)

            # aggregate p, combine, log_softmax, store
            out_sb = bigp.tile([P, NW * OUT], F32, tag="outsb", bufs=1,
                               name="out_sb")
            p_lo = p_full[:, :]
            p_hi = p_full[HALF:N, :]
            for b in range(NBLK):
                nw_b = min(WPB, NW - b * WPB)
                gvs, ohvs = gather_block(b, OUTP, p_lo, p_hi, "L3", F16)
                for wi in range(nw_b):
                    wg = b * WPB + wi
                    rows = P if wg < NW - 1 else LASTW
                    aggw = segsum_window(b, wi, OUTP, gvs, ohvs, "L3")
                    pst = ptr.tile([P, P], F32, tag="tr", name=f"ptrz_{wg}")
                    nc.tensor.transpose(pst[:rows, :OUT],
                                        z3rT[:OUT, wg * P : wg * P + rows],
                                        ident_h[:OUT, :OUT])
                    z3w = agp.tile([P, OUT], F32, tag="agg", name=f"z3w_{wg}")
                    nc.vector.tensor_tensor(out=z3w[:rows, :],
                                            in0=aggw[:rows, :OUT],
                                            in1=pst[:rows, :OUT], op=AL.add)
                    negmax = smp.tile([P, 1], F32, tag="negmax",
                                      name=f"negmax_{wg}")
                    nc.vector.tensor_reduce(out=negmax[:rows, :],
                                            in_=z3w[:rows, :],
                                            axis=mybir.AxisListType.X,
                                            op=AL.max, negate=True)
                    esc = smp.tile([P, OUT], F32, tag="esc", name=f"esc_{wg}")
                    sume = smp.tile([P, 1], F32, tag="sume", name=f"sume_{wg}")
                    nc.scalar.activation(out=esc[:rows, :], in_=z3w[:rows, :],
                                         func=AF.Exp, bias=negmax[:rows, :],
                                         scale=1.0, accum_out=sume[:rows, :])
                    logsum = smp.tile([P, 1], F32, tag="logsum",
                                      name=f"logsum_{wg}")
                    nc.scalar.activation(out=logsum[:rows, :],
                                         in_=sume[:rows, :], func=AF.Ln)
                    nc.vector.tensor_scalar(
                        out=out_sb[:rows, wg * OUT : (wg + 1) * OUT],
                        in0=z3w[:rows, :],
                        scalar1=negmax[:rows, :], scalar2=logsum[:rows, :],
                        op0=AL.add, op1=AL.subtract,
                    )
            # store (full windows in one strided DMA, tail window separately)
            nfull = NW - 1
            nc.sync.dma_start(
                out_d[0 : nfull * P, :].rearrange("(w p) f -> p w f", p=P),
                out_sb[:].rearrange("p (w f) -> p w f", f=OUT)[:, :nfull, :],
            )
            nc.sync.dma_start(
                out_d[nfull * P : S, :],
                out_sb[:LASTW, nfull * OUT : NW * OUT],
            )

    nc.compile()
    return nc


# --------------------------------------------------------------------------
# Entry point
# --------------------------------------------------------------------------

def _make_in_maps(inputs, meta, idx_arrs, dslot_arrs, invd_arrs):
    N = meta["N"]
    S = meta["S"]
    x = np.ascontiguousarray(np.asarray(inputs["x"], dtype=np.float32))
    xg = x.astype(np.float16)
    OUT = np.asarray(inputs["b3"]).shape[0]
    HID = np.asarray(inputs["b1"]).shape[0]

    def bn_pack(v):
        return np.ascontiguousarray(
            np.asarray(v, dtype=np.float32).reshape(2, P).T
        )

    w3l_pad = np.zeros((HID, 128), dtype=np.float16)
    w3l_pad[:, :OUT] = np.asarray(inputs["w3l"], dtype=np.float16)
    b3_pad = np.zeros((P, 1), dtype=np.float32)
    b3_pad[:OUT, 0] = np.asarray(inputs["b3"], dtype=np.float32)

    shared = dict(
        xfull=xg,
        w1l=np.asarray(inputs["w1l"], np.float16),
        w1r=np.asarray(inputs["w1r"], np.float16),
        w2l=np.asarray(inputs["w2l"], np.float16),
        w2r=np.asarray(inputs["w2r"], np.float16),
        w3l=w3l_pad,
        w3r=np.asarray(inputs["w3r"], np.float16),
        g1=bn_pack(inputs["g1"]), be1=bn_pack(inputs["be1"]),
        g2=bn_pack(inputs["g2"]), be2=bn_pack(inputs["be2"]),
        b3=b3_pad,
    )
    in_maps = []
    for c in range(NCORES):
        m = dict(shared)
        m["xT"] = np.ascontiguousarray(xg[c * S : (c + 1) * S, :].T)
        m["idx"] = idx_arrs[c]
        m["dslot"] = dslot_arrs[c]
        m["invd"] = invd_arrs[c]
        in_maps.append(m)
    return in_maps


_CACHE = {}


def _get_compiled(inputs):
    N, INDIM = np.asarray(inputs["x"]).shape
    HID = np.asarray(inputs["b1"]).shape[0]
    OUT = np.asarray(inputs["b3"]).shape[0]
    ei = np.ascontiguousarray(np.asarray(inputs["edge_index"], dtype=np.int64))
    key = (N, INDIM, HID, OUT, hash(ei.tobytes()))
    meta, idx_arrs, dslot_arrs, invd_arrs = preprocess(ei, N)
    if key not in _CACHE:
        _CACHE[key] = build_program(meta, INDIM, HID, OUT)
    return _CACHE[key], meta, idx_arrs, dslot_arrs, invd_arrs


def kernel(**inputs):
    nc, meta, idx_arrs, dslot_arrs, invd_arrs = _get_compiled(inputs)
    in_maps = _make_in_maps(inputs, meta, idx_arrs, dslot_arrs, invd_arrs)
    res = run_bass_kernel_spmd(nc, in_maps, core_ids=list(range(NCORES)))
    return np.concatenate([r["out"] for r in res.results], axis=0)


# revision 38
# speedup vs baseline: 4.6683x; 1.8157x over previous
"""ExpanderSAGE GNN kernel for 8x Trainium2 NeuronCores (Bass/Tile).

Strategy (graph/data parallel, dst-sharded):
  - 50000 nodes sharded 6250/core (8 cores). Each core owns the edges whose
    dst lands in its shard, sorted into (block, half, window) groups where a
    block = 4 consecutive 128-dst windows (512 nodes) and half = src < N/2
    (dma_gather indices are int16, so the node table is split in two halves).
  - Neighbor aggregation: ONE dma_gather per (block, half) fetches all of the
    block's src rows (~4.6k edges) from the node-major table in HBM — few
    SWDGE calls, each with ~300 descriptors (16 idx/descriptor).  A one-hot
    (edge -> dst slot) matrix built with iota+is_equal on DVE turns the
    per-128-edge chunks into PSUM matmul accumulations per window.
  - All dense math in f16 (weights, activations, one-hot, gather tables);
    PSUM accumulation stays f32.  inv_deg scaling fused into PSUM evacuation.
  - BatchNorm: per-core sum / sum-of-squares partials -> AllReduce(8 cores)
    -> affine+ReLU applied via DVE + ACT.  Layer-1 output is rebuilt
    node-major (f16) and AllGather'd to form the next gather table.  Layer 3
    pre-projects p = h2 @ w3l (47 -> pad 128 f16 cols = 256B rows) so only
    256B rows are gathered; log_softmax is computed node-major per window.
"""

import os
import sys

import numpy as np

for _p in ("/opt/trn_rl_repo", os.path.expanduser("~/.axon_site/_ro/trn_rl_repo")):
    if os.path.isdir(_p) and _p not in sys.path:
        sys.path.insert(0, _p)

import concourse.bacc as bacc
import concourse.mybir as mybir
import concourse.tile as tile
from concourse.bass_utils import run_bass_kernel_spmd
from concourse.masks import make_identity

F32 = mybir.dt.float32
F16 = mybir.dt.float16
F8 = mybir.dt.float8e4
I16 = mybir.dt.int16
I32 = mybir.dt.int32
AL = mybir.AluOpType
AF = mybir.ActivationFunctionType

EPS = 1e-5
NCORES = 8
P = 128
BLK = 512  # node block for dense matmuls (= 4 windows)
WPB = BLK // P

# fp8 e4m3 for the layer-2 neighbor table (halves the dominant gather term;
# the self term and all dense math stay f16)
GATHER_F8L2 = os.environ.get("GATHER_F8L2", "0") == "1"
# max indices per dma_gather call (multiple of 128).  With single_packet=True
# the SWDGE packet ceiling is 64 descriptors = 1008 idx -> cap at 896; larger
# calls must use single_packet=False.
GCALL = int(os.environ.get("GCALL", "4608"))
GSP = os.environ.get("GSP", "0") == "1"  # force single_packet=True


# --------------------------------------------------------------------------
# Host-side preprocessing: shard edges, build gather-index / one-hot inputs
# --------------------------------------------------------------------------

def preprocess(edge_index, n_nodes):
    src = np.asarray(edge_index[0], dtype=np.int64)
    dst = np.asarray(edge_index[1], dtype=np.int64)
    S = n_nodes // NCORES
    NW = (S + P - 1) // P
    NB = (NW + WPB - 1) // WPB
    HALF = n_nodes // 2
    assert HALF < 32768 and n_nodes - HALF < 32768

    deg = np.bincount(dst, minlength=n_nodes).astype(np.float32)
    invdeg = (1.0 / np.maximum(deg, 1.0)).astype(np.float32)

    core = dst // S
    local = dst - core * S
    w = local // P
    slot = local % P
    blk = w // WPB
    half = (src >= HALF).astype(np.int64)
    idxval = (src - half * HALF).astype(np.int64)

    # group key (core, block, half, window); stable sort groups the edges
    wib = w - blk * WPB
    key = ((core * NB + blk) * 2 + half) * WPB + wib
    order = np.argsort(key, kind="stable")
    skey = key[order]
    sidx = idxval[order]
    sslot = slot[order]

    ngroups = NCORES * NB * 2 * WPB
    counts = np.bincount(skey, minlength=ngroups).reshape(NCORES, NB, 2, WPB)
    starts = np.zeros(ngroups + 1, dtype=np.int64)
    np.cumsum(counts.reshape(-1), out=starts[1:])

    # uniform chunk counts across cores (SPMD: one NEFF for all 8 cores)
    C = np.ceil(counts.max(axis=0) / P).astype(np.int64)  # [NB, 2, WPB]
    CTOT = int(C.sum())
    LTOT = CTOT * P

    # per-(b,h,w) chunk-column offsets into the concatenated device arrays
    chunk_off = np.zeros((NB, 2, WPB), dtype=np.int64)
    acc = 0
    for b in range(NB):
        for h in range(2):
            for wi in range(WPB):
                chunk_off[b, h, wi] = acc
                acc += C[b, h, wi]

    idx_arrs = []
    dslot_arrs = []
    for c in range(NCORES):
        idx_a = np.zeros((16, LTOT // 16), dtype=np.int16)
        ds_a = np.full((P, CTOT), -1.0, dtype=np.float32)
        for b in range(NB):
            for h in range(2):
                for wi in range(WPB):
                    cg = int(C[b, h, wi])
                    if cg == 0:
                        continue
                    g = ((c * NB + b) * 2 + h) * WPB + wi
                    s0, s1 = starts[g], starts[g + 1]
                    k = s1 - s0
                    Lw = cg * P
                    buf = np.zeros(Lw, dtype=np.int16)
                    buf[:k] = sidx[s0:s1].astype(np.int16)
                    co = int(chunk_off[b, h, wi])
                    idx_a[:, co * 8 : co * 8 + Lw // 16] = buf.reshape(-1, 16).T
                    sl = np.full(Lw, -1.0, dtype=np.float32)
                    sl[:k] = sslot[s0:s1].astype(np.float32)
                    ds_a[:, co : co + cg] = sl.reshape(cg, P).T
        idx_arrs.append(np.tile(idx_a, (8, 1)))  # replicate to 128 partitions
        dslot_arrs.append(ds_a.astype(np.float16))

    # invdeg per core, [128, NW] (partition = slot, col = window), pad 1.0
    invd_arrs = []
    for c in range(NCORES):
        v = np.ones(NW * P, dtype=np.float32)
        v[:S] = invdeg[c * S : (c + 1) * S]
        invd_arrs.append(v.reshape(NW, P).T.copy())

    # ---- layer-3 paired-table grouping: pair row = src//2, no half split
    key3 = (core * NB + blk) * WPB + wib
    order3 = np.argsort(key3, kind="stable")
    skey3 = key3[order3]
    sidx3 = (src // 2)[order3]
    spar3 = (src % 2)[order3]
    sslot3 = slot[order3]

    ngroups3 = NCORES * NB * WPB
    counts3 = np.bincount(skey3, minlength=ngroups3).reshape(NCORES, NB, WPB)
    starts3 = np.zeros(ngroups3 + 1, dtype=np.int64)
    np.cumsum(counts3.reshape(-1), out=starts3[1:])
    C3 = np.ceil(counts3.max(axis=0) / P).astype(np.int64)  # [NB, WPB]
    CTOT3 = int(C3.sum())
    LTOT3 = CTOT3 * P
    chunk_off3 = np.zeros((NB, WPB), dtype=np.int64)
    acc = 0
    for b in range(NB):
        for wi in range(WPB):
            chunk_off3[b, wi] = acc
            acc += C3[b, wi]

    idx3_arrs, dse_arrs, dso_arrs = [], [], []
    for c in range(NCORES):
        idx_a = np.zeros((16, LTOT3 // 16), dtype=np.int16)
        de_a = np.full((P, CTOT3), -1.0, dtype=np.float32)
        do_a = np.full((P, CTOT3), -1.0, dtype=np.float32)
        for b in range(NB):
            for wi in range(WPB):
                cg = int(C3[b, wi])
                if cg == 0:
                    continue
                g = (c * NB + b) * WPB + wi
                s0, s1 = starts3[g], starts3[g + 1]
                k = s1 - s0
                Lw = cg * P
                buf = np.zeros(Lw, dtype=np.int16)
                buf[:k] = sidx3[s0:s1].astype(np.int16)
                co = int(chunk_off3[b, wi])
                idx_a[:, co * 8 : co * 8 + Lw // 16] = buf.reshape(-1, 16).T
                se = np.full(Lw, -1.0, dtype=np.float32)
                so = np.full(Lw, -1.0, dtype=np.float32)
                par = spar3[s0:s1]
                sl = sslot3[s0:s1].astype(np.float32)
                se[:k] = np.where(par == 0, sl, -1.0)
                so[:k] = np.where(par == 1, sl, -1.0)
                de_a[:, co : co + cg] = se.reshape(cg, P).T
                do_a[:, co : co + cg] = so.reshape(cg, P).T
        idx3_arrs.append(np.tile(idx_a, (8, 1)))
        dse_arrs.append(de_a.astype(np.float16))
        dso_arrs.append(do_a.astype(np.float16))

    meta = dict(N=n_nodes, S=S, NW=NW, NB=NB, HALF=HALF, C=C,
                chunk_off=chunk_off, CTOT=CTOT, ITOT=LTOT // 16,
                C3=C3, chunk_off3=chunk_off3, CTOT3=CTOT3,
                ITOT3=LTOT3 // 16)
    return (meta, idx_arrs, dslot_arrs, invd_arrs,
            idx3_arrs, dse_arrs, dso_arrs)


# --------------------------------------------------------------------------
# Device program
# --------------------------------------------------------------------------

def build_program(meta, INDIM, HID, OUT, reps=1, ncores=NCORES, mock_cc=False,
                  abl=()):
    N, S, NW, NB, HALF = (meta["N"], meta["S"], meta["NW"], meta["NB"],
                          meta["HALF"])
    C, chunk_off, CTOT, ITOT = (meta["C"], meta["chunk_off"], meta["CTOT"],
                                meta["ITOT"])
    C3, chunk_off3, CTOT3, ITOT3 = (meta["C3"], meta["chunk_off3"],
                                    meta["CTOT3"], meta["ITOT3"])
    LASTW = S - P * (NW - 1)
    NBLK = (S + BLK - 1) // BLK
    assert NBLK == NB
    OUTP = 64  # padded projection width for layer 3 (47 -> 64; 2-node pairs
    #            packed per 256B table row)
    RG = [list(range(ncores))]

    nc = bacc.Bacc("TRN2", target_bir_lowering=False, debug=False,
                   num_devices=ncores, num_swdge_queues=4)
    qctr = [0]

    # ---- I/O ----
    L2DT = F8 if GATHER_F8L2 else F16
    xfull = nc.dram_tensor("xfull", [N, INDIM], F16, kind="ExternalInput")
    xT = nc.dram_tensor("xT", [P, S], F16, kind="ExternalInput")
    idx_d = nc.dram_tensor("idx", [P, ITOT], I16, kind="ExternalInput")
    dslot_d = nc.dram_tensor("dslot", [P, CTOT], F16, kind="ExternalInput")
    idx3_d = nc.dram_tensor("idx3", [P, ITOT3], I16, kind="ExternalInput")
    dse_d = nc.dram_tensor("dse", [P, CTOT3], F16, kind="ExternalInput")
    dso_d = nc.dram_tensor("dso", [P, CTOT3], F16, kind="ExternalInput")
    invd_d = nc.dram_tensor("invd", [P, NW], F32, kind="ExternalInput")
    w1l_d = nc.dram_tensor("w1l", [INDIM, HID], F16, kind="ExternalInput")
    w1r_d = nc.dram_tensor("w1r", [INDIM, HID], F16, kind="ExternalInput")
    w2l_d = nc.dram_tensor("w2l", [HID, HID], F16, kind="ExternalInput")
    w2r_d = nc.dram_tensor("w2r", [HID, HID], F16, kind="ExternalInput")
    w3l_d = nc.dram_tensor("w3l", [HID, OUTP], F16, kind="ExternalInput")
    w3r_d = nc.dram_tensor("w3r", [HID, OUT], F16, kind="ExternalInput")
    g1_d = nc.dram_tensor("g1", [P, 2], F32, kind="ExternalInput")
    be1_d = nc.dram_tensor("be1", [P, 2], F32, kind="ExternalInput")
    g2_d = nc.dram_tensor("g2", [P, 2], F32, kind="ExternalInput")
    be2_d = nc.dram_tensor("be2", [P, 2], F32, kind="ExternalInput")
    b3_d = nc.dram_tensor("b3", [P, 1], F32, kind="ExternalInput")
    out_d = nc.dram_tensor("out", [S, OUT], F32, kind="ExternalOutput")

    from contextlib import ExitStack

    with tile.TileContext(nc) as tc, ExitStack() as es:
        cp = es.enter_context(tc.tile_pool(name="const", bufs=1))
        gp = es.enter_context(tc.tile_pool(name="gath", bufs=3))
        ohp = es.enter_context(tc.tile_pool(name="oh", bufs=2))
        agp = es.enter_context(tc.tile_pool(name="agg", bufs=4))
        atp = es.enter_context(tc.tile_pool(name="aggT", bufs=4))
        bigp = es.enter_context(tc.tile_pool(name="big", bufs=4))
        sqp = es.enter_context(tc.tile_pool(name="sq", bufs=1))
        smp = es.enter_context(tc.tile_pool(name="small", bufs=2))
        pseg = es.enter_context(tc.tile_pool(name="pseg", bufs=3, space="PSUM"))
        ptr = es.enter_context(tc.tile_pool(name="ptr", bufs=2, space="PSUM"))
        pz = es.enter_context(tc.tile_pool(name="pz", bufs=2, space="PSUM"))
        drp = es.enter_context(tc.tile_pool(name="dram", bufs=1, space="DRAM"))

        # ---- constants ----
        idx_sb = cp.tile([P, ITOT], I16, name="idx_sb")
        nc.sync.dma_start(idx_sb[:], idx_d[:, :])
        dslot_h = cp.tile([P, CTOT], F16, name="dslot_h")
        nc.sync.dma_start(dslot_h[:], dslot_d[:, :])
        idx3_sb = cp.tile([P, ITOT3], I16, name="idx3_sb")
        nc.sync.dma_start(idx3_sb[:], idx3_d[:, :])
        dse_h = cp.tile([P, CTOT3], F16, name="dse_h")
        nc.sync.dma_start(dse_h[:], dse_d[:, :])
        dso_h = cp.tile([P, CTOT3], F16, name="dso_h")
        nc.sync.dma_start(dso_h[:], dso_d[:, :])
        invd_sb = cp.tile([P, NW], F32, name="invd_sb")
        nc.sync.dma_start(invd_sb[:], invd_d[:, :])

        w1l_sb = cp.tile([P, HID], F16, name="w1l_sb")
        nc.sync.dma_start(w1l_sb[:], w1l_d[:, :])
        w1r_sb = cp.tile([P, HID], F16, name="w1r_sb")
        nc.sync.dma_start(w1r_sb[:], w1r_d[:, :])
        w2l_sb = [cp.tile([P, HID], F16, name=f"w2l_sb{k}") for k in range(2)]
        w2r_sb = [cp.tile([P, HID], F16, name=f"w2r_sb{k}") for k in range(2)]
        w3l_sb = [cp.tile([P, OUTP], F16, name=f"w3l_sb{k}") for k in range(2)]
        w3r_sb = [cp.tile([P, OUT], F16, name=f"w3r_sb{k}") for k in range(2)]
        for k in range(2):
            nc.sync.dma_start(w2l_sb[k][:], w2l_d[k * P : (k + 1) * P, :])
            nc.sync.dma_start(w2r_sb[k][:], w2r_d[k * P : (k + 1) * P, :])
            nc.sync.dma_start(w3l_sb[k][:], w3l_d[k * P : (k + 1) * P, :])
            nc.sync.dma_start(w3r_sb[k][:], w3r_d[k * P : (k + 1) * P, :])
        g1_sb = cp.tile([P, 2], F32, name="g1_sb")
        nc.sync.dma_start(g1_sb[:], g1_d[:, :])
        be1_sb = cp.tile([P, 2], F32, name="be1_sb")
        nc.sync.dma_start(be1_sb[:], be1_d[:, :])
        g2_sb = cp.tile([P, 2], F32, name="g2_sb")
        nc.sync.dma_start(g2_sb[:], g2_d[:, :])
        be2_sb = cp.tile([P, 2], F32, name="be2_sb")
        nc.sync.dma_start(be2_sb[:], be2_d[:, :])
        b3_sb = cp.tile([P, 1], F32, name="b3_sb")
        nc.sync.dma_start(b3_sb[:], b3_d[:, :])

        iota_i = cp.tile([P, P], I32, name="iota_i")
        nc.gpsimd.iota(iota_i[:], pattern=[[1, P]], base=0, channel_multiplier=0)
        iota_h = cp.tile([P, P], F16, name="iota_h")
        nc.vector.tensor_copy(iota_h[:], iota_i[:])
        ident_h = cp.tile([P, P], F16, name="ident_h")
        make_identity(nc, ident_h[:])
        ident_f = cp.tile([P, P], F32, name="ident_f")
        make_identity(nc, ident_f[:])
        eps_sb = cp.tile([P, 1], F32, name="eps_sb")
        nc.vector.memset(eps_sb[:], EPS)

        if GATHER_F8L2:
            iota_8 = cp.tile([P, P], F8, name="iota_8")
            nc.vector.tensor_copy(iota_8[:], iota_i[:])
            dslot_8 = cp.tile([P, CTOT], F8, name="dslot_8")
            nc.vector.tensor_copy(dslot_8[:], dslot_h[:])

        def gather_block(b, F, lo_ap, hi_ap, lname, tdt):
            """One dma_gather + one-hot build per (block, half). Returns
            (gv, ohv) lists indexed by half, each [128, CB_h, F/P-dims]."""
            gvs, ohvs = [], []
            for h, base_ap in ((0, lo_ap), (1, hi_ap)):
                cb = int(C[b, h, :].sum())
                if cb == 0:
                    gvs.append(None)
                    ohvs.append(None)
                    continue
                co = int(chunk_off[b, h, 0])
                g_t = gp.tile([P, cb * F], tdt, tag="g", name=f"g{lname}_{b}_{h}")
                gv = g_t[:].rearrange("p (c f) -> p c f", f=F)
                maxc = max(1, GCALL // P)
                npieces = -(-cb // maxc)
                sizes = [cb // npieces + (1 if i < cb % npieces else 0)
                         for i in range(npieces)]
                offs = [sum(sizes[:i]) for i in range(npieces)]
                if "nogather" in abl:
                    nc.vector.memset(g_t[:, 0:1], 0.0)
                else:
                    for c0, cn in zip(offs, sizes):
                        nc.gpsimd.dma_gather(
                            out_ap=gv[:, c0 : c0 + cn, :],
                            in_ap=base_ap,
                            idxs_ap=idx_sb[:, (co + c0) * 8 : (co + c0 + cn) * 8],
                            num_idxs=cn * P,
                            num_idxs_reg=cn * P,
                            elem_size=F,
                            single_packet=GSP or (cn * P <= 896),
                            queue_num=qctr[0] % 4,
                        )
                        qctr[0] += 1
                oh_t = ohp.tile([P, cb * P], tdt, tag="oh",
                                name=f"oh{lname}_{b}_{h}")
                ohv = oh_t[:].rearrange("p (c q) -> p c q", q=P)
                if "noonehot" in abl:
                    nc.vector.memset(oh_t[:, 0:1], 0.0)
                else:
                    iota_t = iota_8 if tdt == F8 else iota_h
                    dslot_t = dslot_8 if tdt == F8 else dslot_h
                    nc.vector.tensor_tensor(
                        out=ohv,
                        in0=iota_t[:].unsqueeze(1).to_broadcast([P, cb, P]),
                        in1=dslot_t[:, co : co + cb].unsqueeze(2).to_broadcast(
                            [P, cb, P]
                        ),
                        op=AL.is_equal,
                    )
                gvs.append(gv)
                ohvs.append(ohv)
            return gvs, ohvs

        def segsum_window(b, wi, F, gvs, ohvs, lname, adt=F16):
            """Accumulate window wi of block b from the block's gathered
            tiles; returns scaled agg tile [128, F] (partition=dst)."""
            wg = b * WPB + wi
            ctot = int(C[b, :, wi].sum())
            aggw = agp.tile([P, F], adt, tag="agg", name=f"agg{lname}_{wg}")
            if ctot == 0:
                nc.vector.memset(aggw[:], 0.0)
                return aggw
            ps = pseg.tile([P, F], F32, tag="seg", name=f"pseg{lname}_{wg}")
            done = 0
            for h in range(2):
                cg = int(C[b, h, wi])
                if cg == 0:
                    continue
                lo = int(chunk_off[b, h, wi] - chunk_off[b, h, 0])
                for ch in range(lo, lo + cg):
                    if "nosegmm" in abl:
                        done += 1
                        continue
                    nc.tensor.matmul(
                        out=ps[:],
                        lhsT=ohvs[h][:, ch, :],
                        rhs=gvs[h][:, ch, :],
                        start=(done == 0),
                        stop=(done == ctot - 1),
                    )
                    done += 1
            nc.vector.tensor_scalar(
                out=aggw[:], in0=ps[:], scalar1=invd_sb[:, wg : wg + 1],
                scalar2=None, op0=AL.mult,
            )
            return aggw

        def gather_block3(b, table_ap):
            """L3: one dma_gather per block from the paired p table (single
            base, no half split) + even/odd one-hot builds."""
            cb = int(C3[b, :].sum())
            co = int(chunk_off3[b, 0])
            g_t = gp.tile([P, cb * P], F16, tag="g", name=f"gL3_{b}")
            gv = g_t[:].rearrange("p (c f) -> p c f", f=P)
            maxc = max(1, GCALL // P)
            npieces = -(-cb // maxc)
            sizes = [cb // npieces + (1 if i < cb % npieces else 0)
                     for i in range(npieces)]
            offs = [sum(sizes[:i]) for i in range(npieces)]
            for c0, cn in zip(offs, sizes):
                nc.gpsimd.dma_gather(
                    out_ap=gv[:, c0 : c0 + cn, :],
                    in_ap=table_ap,
                    idxs_ap=idx3_sb[:, (co + c0) * 8 : (co + c0 + cn) * 8],
                    num_idxs=cn * P,
                    num_idxs_reg=cn * P,
                    elem_size=P,
                    single_packet=GSP or (cn * P <= 896),
                    queue_num=qctr[0] % 4,
                )
                qctr[0] += 1
            ohs = []
            for par, dsl in ((0, dse_h), (1, dso_h)):
                oh_t = ohp.tile([P, cb * P], F16, tag="oh",
                                name=f"ohL3_{b}_{par}")
                ohv = oh_t[:].rearrange("p (c q) -> p c q", q=P)
                nc.vector.tensor_tensor(
                    out=ohv,
                    in0=iota_h[:].unsqueeze(1).to_broadcast([P, cb, P]),
                    in1=dsl[:, co : co + cb].unsqueeze(2).to_broadcast(
                        [P, cb, P]),
                    op=AL.is_equal,
                )
                ohs.append(ohv)
            return gv, ohs

        def segsum_window3(b, wi, gv, ohs):
            """L3 window aggregation: even/odd halves of each gathered pair
            row, accumulated into one [128, 64] psum."""
            wg = b * WPB + wi
            cg = int(C3[b, wi])
            aggw = agp.tile([P, OUTP], F16, tag="agg", name=f"aggL3_{wg}")
            if cg == 0:
                nc.vector.memset(aggw[:], 0.0)
                return aggw
            ps = pseg.tile([P, OUTP], F32, tag="seg", name=f"psegL3_{wg}")
            lo = int(chunk_off3[b, wi] - chunk_off3[b, 0])
            nmm = 2 * cg
            k = 0
            for ch in range(lo, lo + cg):
                for par in range(2):
                    nc.tensor.matmul(
                        out=ps[:, :],
                        lhsT=ohs[par][:, ch, :],
                        rhs=gv[:, ch, par * OUTP : (par + 1) * OUTP],
                        start=(k == 0),
                        stop=(k == nmm - 1),
                    )
                    k += 1
            nc.vector.tensor_scalar(
                out=aggw[:], in0=ps[:], scalar1=invd_sb[:, wg : wg + 1],
                scalar2=None, op0=AL.mult,
            )
            return aggw

        def dense_layer(lname, F_in, lo_ap, hi_ap, hT, wl_sb, wr_sb, tdt):
            """Full SAGE layer (aggregate + dense), feature-major output.
            Returns (zT halves f16, sum partials, sumsq partials)."""
            nh_in = F_in // P
            zT = [bigp.tile([P, S], F16, tag="big", name=f"zT{lname}_{m}")
                  for m in range(2)]
            szp = [smp.tile([P, NBLK], F32, tag=f"szp{lname}{m}",
                            name=f"szp{lname}{m}") for m in range(2)]
            ssqp = [smp.tile([P, NBLK], F32, tag=f"ssqp{lname}{m}",
                             name=f"ssqp{lname}{m}") for m in range(2)]
            for b in range(NBLK):
                n0 = b * BLK
                nb = min(S, n0 + BLK) - n0
                nw_b = min(WPB, NW - b * WPB)
                gvs, ohvs = gather_block(b, F_in, lo_ap, hi_ap, lname, tdt)
                aggT = [atp.tile([P, BLK], F16, tag="aggT",
                                 name=f"aggT{lname}_{b}_{h}")
                        for h in range(nh_in)]
                for wi in range(nw_b):
                    aggw = segsum_window(b, wi, F_in, gvs, ohvs, lname)
                    for h in range(nh_in):
                        pst = ptr.tile([P, P], F16, tag="tr",
                                       name=f"ptr{lname}_{b}_{wi}_{h}")
                        nc.tensor.transpose(
                            pst[:], aggw[:, h * P : (h + 1) * P], ident_h[:]
                        )
                        c0 = wi * P
                        cw = min(P, nb - c0)
                        nc.vector.tensor_copy(
                            out=aggT[h][:, c0 : c0 + cw], in_=pst[:, :cw]
                        )
                for m in range(2):
                    psz = pz.tile([P, BLK], F32, tag="z",
                                  name=f"pz{lname}_{b}_{m}")
                    mcols = slice(m * P, (m + 1) * P)
                    nmm = 2 * nh_in
                    k = 0
                    for h in range(nh_in):
                        nc.tensor.matmul(
                            out=psz[:, :nb],
                            lhsT=wl_sb[h][:, mcols],
                            rhs=aggT[h][:, :nb],
                            start=(k == 0), stop=(k == nmm - 1),
                        )
                        k += 1
                    for h in range(nh_in):
                        nc.tensor.matmul(
                            out=psz[:, :nb],
                            lhsT=wr_sb[h][:, mcols],
                            rhs=hT[h][:, n0 : n0 + nb],
                            start=(k == 0), stop=(k == nmm - 1),
                        )
                        k += 1
                    nc.vector.tensor_reduce(
                        out=szp[m][:, b : b + 1], in_=psz[:, :nb],
                        axis=mybir.AxisListType.X, op=AL.add,
                    )
                    sqsc = sqp.tile([P, BLK], F32, tag="sq",
                                    name=f"sq{lname}_{b}_{m}")
                    nc.scalar.activation(
                        out=sqsc[:, :nb], in_=psz[:, :nb], func=AF.Square,
                        accum_out=ssqp[m][:, b : b + 1],
                    )
                    nc.vector.tensor_copy(out=zT[m][:, n0 : n0 + nb],
                                          in_=psz[:, :nb])
            return zT, szp, ssqp

        def collective(kind, op, ins, outs):
            if mock_cc:
                nc.sync.dma_start(outs[0][0 : ins[0].shape[0]], ins[0])
            else:
                nc.gpsimd.collective_compute(kind, op, replica_groups=RG,
                                             ins=ins, outs=outs)

        def bn_relu(li, lname, zT, szp, ssqp, g_sb, be_sb):
            """AllReduce stats, then hT = relu((z - mean) * a + be), f16."""
            stat = smp.tile([P, 4], F32, tag=f"stat{lname}", name=f"stat{lname}")
            for m in range(2):
                nc.vector.tensor_reduce(out=stat[:, m : m + 1], in_=szp[m][:],
                                        axis=mybir.AxisListType.X, op=AL.add)
                nc.vector.tensor_reduce(out=stat[:, 2 + m : 3 + m],
                                        in_=ssqp[m][:],
                                        axis=mybir.AxisListType.X, op=AL.add)
            nc.sync.dma_start(st_in[li][:], stat[:])
            collective("AllGather", AL.bypass, [st_in[li][:]], [st_out[li][:]])
            stat8 = smp.tile([P, 4 * ncores], F32, tag=f"stat8{lname}",
                             name=f"stat8{lname}")
            nc.sync.dma_start(
                stat8[:].rearrange("p (s c) -> p s c", c=ncores),
                st_out[li][:, :].rearrange("(c p) s -> p s c", p=P),
            )
            statg = smp.tile([P, 4], F32, tag=f"statg{lname}", name=f"statg{lname}")
            nc.vector.tensor_reduce(
                out=statg[:].unsqueeze(2),
                in_=stat8[:].rearrange("p (s c) -> p s c", c=ncores),
                axis=mybir.AxisListType.X, op=AL.add,
            )
            hT = []
            for m in range(2):
                mean = smp.tile([P, 1], F32, tag=f"mean{lname}{m}",
                                name=f"mean{lname}{m}")
                nc.vector.tensor_scalar(out=mean[:], in0=statg[:, m : m + 1],
                                        scalar1=1.0 / N, scalar2=None,
                                        op0=AL.mult)
                ex2 = smp.tile([P, 1], F32, tag=f"ex2{lname}{m}",
                               name=f"ex2{lname}{m}")
                nc.vector.tensor_scalar(out=ex2[:], in0=statg[:, 2 + m : 3 + m],
                                        scalar1=1.0 / N, scalar2=None,
                                        op0=AL.mult)
                nvar = smp.tile([P, 1], F32, tag=f"nvar{lname}{m}",
                                name=f"nvar{lname}{m}")
                # nvar = mean^2 - E[x^2]  (= -var)
                nc.vector.scalar_tensor_tensor(
                    out=nvar[:], in0=mean[:], scalar=mean[:], in1=ex2[:],
                    op0=AL.mult, op1=AL.subtract,
                )
                std = smp.tile([P, 1], F32, tag=f"std{lname}{m}",
                               name=f"std{lname}{m}")
                nc.scalar.activation(out=std[:], in_=nvar[:], func=AF.Sqrt,
                                     bias=eps_sb[:], scale=-1.0)
                istd = smp.tile([P, 1], F32, tag=f"istd{lname}{m}",
                                name=f"istd{lname}{m}")
                nc.vector.reciprocal(istd[:], std[:])
                a_m = smp.tile([P, 1], F32, tag=f"a{lname}{m}",
                               name=f"a{lname}{m}")
                nc.vector.tensor_tensor(out=a_m[:], in0=g_sb[:, m : m + 1],
                                        in1=istd[:], op=AL.mult)
                h_m = bigp.tile([P, S], F16, tag="big", name=f"hT{lname}_{m}")
                nc.vector.tensor_scalar(out=h_m[:], in0=zT[m][:],
                                        scalar1=mean[:], scalar2=a_m[:],
                                        op0=AL.subtract, op1=AL.mult)
                nc.scalar.activation(out=h_m[:], in_=h_m[:], func=AF.Relu,
                                     bias=be_sb[:, m : m + 1], scale=1.0)
                hT.append(h_m)
            return hT

        for rep in range(reps):
            # ---- DRAM intermediates (fresh per rep: Shared tiles are
            # single-writer) ----
            h1_shard = drp.tile([S, HID], L2DT, name=f"h1_shard_{rep}")
            h1_full = drp.tile([N, HID], L2DT, name=f"h1_full_{rep}",
                               addr_space="Shared")
            p_shard = drp.tile([S // 2, P], F16, name=f"p_shard_{rep}")
            p_full = drp.tile([N // 2, P], F16, name=f"p_full_{rep}",
                              addr_space="Shared")
            st_in = [drp.tile([P, 4], F32, name=f"st_in{l}_{rep}")
                     for l in range(2)]
            st_out = [drp.tile([P * ncores, 4], F32, name=f"st_out{l}_{rep}",
                               addr_space="Shared")
                      for l in range(2)]
            # ================= Layer 1 =================
            xT_sb = bigp.tile([P, S], F16, tag="big", name="xT_sb")
            nc.sync.dma_start(xT_sb[:], xT[:, :])
            x_lo = xfull[:, :]
            x_hi = xfull[HALF:N, :]
            zT1, szp1, ssqp1 = dense_layer("L1", INDIM, x_lo, x_hi, [xT_sb],
                                           [w1l_sb], [w1r_sb], tdt=F16)
            h1T = bn_relu(0, "L1", zT1, szp1, ssqp1, g1_sb, be1_sb)

            # rebuild node-major h1 and AllGather the full table
            for wi in range(NW):
                rows = P if wi < NW - 1 else LASTW
                hnm = agp.tile([P, HID], L2DT, tag="agg", name=f"hnm_{wi}")
                for h in range(2):
                    pst = ptr.tile([P, P], F16, tag="tr", name=f"ptrh_{wi}_{h}")
                    nc.tensor.transpose(pst[:rows, :],
                                        h1T[h][:, wi * P : wi * P + rows],
                                        ident_h[:])
                    nc.vector.tensor_copy(out=hnm[:rows, h * P : (h + 1) * P],
                                          in_=pst[:rows, :])
                nc.sync.dma_start(h1_shard[wi * P : wi * P + rows, :],
                                  hnm[:rows, :])
            collective("AllGather", AL.bypass, [h1_shard[:, :]], [h1_full[:, :]])

            # ================= Layer 2 =================
            h1_lo = h1_full[:, :]
            h1_hi = h1_full[HALF:N, :]
            zT2, szp2, ssqp2 = dense_layer("L2", HID, h1_lo, h1_hi, h1T,
                                           w2l_sb, w2r_sb, tdt=L2DT)
            h2T = bn_relu(1, "L2", zT2, szp2, ssqp2, g2_sb, be2_sb)

            # ================= Layer 3 =================
            # p = h2 @ w3l (padded to 64 f16 cols), packed 2 nodes per
            # 256B table row, rebuilt pair-major + AG
            pT = bigp.tile([P, S], F16, tag="big", name="pT")
            for b in range(NBLK):
                n0 = b * BLK
                nb = min(S, n0 + BLK) - n0
                psp = pz.tile([P, BLK], F32, tag="z", name=f"pzp_{b}")
                for h in range(2):
                    nc.tensor.matmul(
                        out=psp[:OUTP, :nb], lhsT=w3l_sb[h][:],
                        rhs=h2T[h][:, n0 : n0 + nb],
                        start=(h == 0), stop=(h == 1),
                    )
                nc.vector.tensor_copy(out=pT[:OUTP, n0 : n0 + nb],
                                      in_=psp[:OUTP, :nb])
            pTv = pT[:OUTP, :].rearrange("f (k t) -> f k t", t=2)
            for wi in range(NW):
                rows = P if wi < NW - 1 else LASTW
                prs = rows // 2
                pnm = agp.tile([P, P], F16, tag="agg", name=f"pnm_{wi}")
                for par in range(2):
                    pst = ptr.tile([P, P], F16, tag="tr",
                                   name=f"ptrp_{wi}_{par}")
                    nc.tensor.transpose(
                        pst[:prs, :OUTP],
                        pTv[:, wi * 64 : wi * 64 + prs, par],
                        ident_h[:OUTP, :OUTP])
                    nc.vector.tensor_copy(
                        out=pnm[:prs, par * OUTP : (par + 1) * OUTP],
                        in_=pst[:prs, :OUTP])
                nc.sync.dma_start(p_shard[wi * 64 : wi * 64 + prs, :],
                                  pnm[:prs, :])
            collective("AllGather", AL.bypass, [p_shard[:, :]], [p_full[:, :]])

            # z3r^T = w3r^T @ h2^T + b3 (feature-major, 47 rows)
            z3rT = bigp.tile([P, S], F16, tag="bigz", bufs=1, name="z3rT")
            for b in range(NBLK):
                n0 = b * BLK
                nb = min(S, n0 + BLK) - n0
                psr = pz.tile([P, BLK], F32, tag="z", name=f"pzr_{b}")
                for h in range(2):
                    nc.tensor.matmul(
                        out=psr[:OUT, :nb], lhsT=w3r_sb[h][:],
                        rhs=h2T[h][:, n0 : n0 + nb],
                        start=(h == 0), stop=(h == 1),
                    )
                nc.vector.tensor_scalar(out=z3rT[:OUT, n0 : n0 + nb],
                                        in0=psr[:OUT, :nb],
                                        scalar1=b3_sb[:OUT, :], scalar2=None,
                                        op0=AL.add)

            # aggregate p, combine, log_softmax, store per window
            for b in range(NBLK):
                nw_b = min(WPB, NW - b * WPB)
                gv3, ohs3 = gather_block3(b, p_full[:, :])
                for wi in range(nw_b):
                    wg = b * WPB + wi
                    rows = P if wg < NW - 1 else LASTW
                    aggw = segsum_window3(b, wi, gv3, ohs3)
                    pst = ptr.tile([P, P], F16, tag="tr", name=f"ptrz_{wg}")
                    nc.tensor.transpose(pst[:rows, :OUT],
                                        z3rT[:OUT, wg * P : wg * P + rows],
                                        ident_h[:OUT, :OUT])
                    z3w = agp.tile([P, OUT], F32, tag="agg", name=f"z3w_{wg}")
                    nc.vector.tensor_tensor(out=z3w[:rows, :],
                                            in0=aggw[:rows, :OUT],
                                            in1=pst[:rows, :OUT], op=AL.add)
                    negmax = smp.tile([P, 1], F32, tag="negmax",
                                      name=f"negmax_{wg}")
                    nc.vector.tensor_reduce(out=negmax[:rows, :],
                                            in_=z3w[:rows, :],
                                            axis=mybir.AxisListType.X,
                                            op=AL.max, negate=True)
                    esc = smp.tile([P, OUT], F32, tag="esc", name=f"esc_{wg}")
                    sume = smp.tile([P, 1], F32, tag="sume", name=f"sume_{wg}")
                    nc.scalar.activation(out=esc[:rows, :], in_=z3w[:rows, :],
                                         func=AF.Exp, bias=negmax[:rows, :],
                                         scale=1.0, accum_out=sume[:rows, :])
                    logsum = smp.tile([P, 1], F32, tag="logsum",
                                      name=f"logsum_{wg}")
                    nc.scalar.activation(out=logsum[:rows, :],
                                         in_=sume[:rows, :], func=AF.Ln)
                    ow = smp.tile([P, OUT], F32, tag="ow", name=f"ow_{wg}")
                    nc.vector.tensor_scalar(
                        out=ow[:rows, :],
                        in0=z3w[:rows, :],
                        scalar1=negmax[:rows, :], scalar2=logsum[:rows, :],
                        op0=AL.add, op1=AL.subtract,
                    )
                    nc.sync.dma_start(out_d[wg * P : wg * P + rows, :],
                                      ow[:rows, :])

    nc.compile()
    return nc


# --------------------------------------------------------------------------
# Entry point
# --------------------------------------------------------------------------

def _make_in_maps(inputs, meta, *arrs):
    (idx_arrs, dslot_arrs, invd_arrs, idx3_arrs, dse_arrs, dso_arrs) = arrs
    N = meta["N"]
    S = meta["S"]
    x = np.ascontiguousarray(np.asarray(inputs["x"], dtype=np.float32))
    xg = x.astype(np.float16)
    OUT = np.asarray(inputs["b3"]).shape[0]
    HID = np.asarray(inputs["b1"]).shape[0]

    def bn_pack(v):
        return np.ascontiguousarray(
            np.asarray(v, dtype=np.float32).reshape(2, P).T
        )

    w3l_pad = np.zeros((HID, 64), dtype=np.float16)
    w3l_pad[:, :OUT] = np.asarray(inputs["w3l"], dtype=np.float16)
    b3_pad = np.zeros((P, 1), dtype=np.float32)
    b3_pad[:OUT, 0] = np.asarray(inputs["b3"], dtype=np.float32)

    shared = dict(
        xfull=xg,
        w1l=np.asarray(inputs["w1l"], np.float16),
        w1r=np.asarray(inputs["w1r"], np.float16),
        w2l=np.asarray(inputs["w2l"], np.float16),
        w2r=np.asarray(inputs["w2r"], np.float16),
        w3l=w3l_pad,
        w3r=np.asarray(inputs["w3r"], np.float16),
        g1=bn_pack(inputs["g1"]), be1=bn_pack(inputs["be1"]),
        g2=bn_pack(inputs["g2"]), be2=bn_pack(inputs["be2"]),
        b3=b3_pad,
    )
    in_maps = []
    for c in range(NCORES):
        m = dict(shared)
        m["xT"] = np.ascontiguousarray(xg[c * S : (c + 1) * S, :].T)
        m["idx"] = idx_arrs[c]
        m["dslot"] = dslot_arrs[c]
        m["invd"] = invd_arrs[c]
        m["idx3"] = idx3_arrs[c]
        m["dse"] = dse_arrs[c]
        m["dso"] = dso_arrs[c]
        in_maps.append(m)
    return in_maps


_CACHE = {}


def _get_compiled(inputs):
    N, INDIM = np.asarray(inputs["x"]).shape
    HID = np.asarray(inputs["b1"]).shape[0]
    OUT = np.asarray(inputs["b3"]).shape[0]
    ei = np.ascontiguousarray(np.asarray(inputs["edge_index"], dtype=np.int64))
    key = (N, INDIM, HID, OUT, hash(ei.tobytes()))
    pre = preprocess(ei, N)
    if key not in _CACHE:
        _CACHE[key] = build_program(pre[0], INDIM, HID, OUT)
    return (_CACHE[key],) + pre


def kernel(**inputs):
    nc, meta, *arrs = _get_compiled(inputs)
    in_maps = _make_in_maps(inputs, meta, *arrs)
    res = run_bass_kernel_spmd(nc, in_maps, core_ids=list(range(NCORES)))
    return np.concatenate([r["out"] for r in res.results], axis=0)
